# revision 26
# baseline (speedup 1.0000x reference)
"""Trainium2 Bass kernel for nn_Attn_58669253263845 (sparse_attention).

Reference computation:
    hidden2 = concat(hidden[0], hidden[1])                 # [B, 2H]
    attn_input = concat(bcast(hidden2), encoder_outputs)   # [B, S, 3H]
    energy = attn_input @ W.T + b                          # [B, S, H]
    scores = energy @ v                                    # [B, S]
    out = softmax(scores, axis=S)

Everything before the softmax is linear, so
    scores[b,s] = attn_input[b,s,:] . (v @ W) + v.b
                = hidden2[b,:] . w_hid + enc[b,s,:] . w_enc + v.b
The hidden/bias terms are constant per batch row and cancel in the softmax
over S.  Hence:
    out = softmax_s(enc[b,s,:] . w_enc),  w_enc = v @ W[:, 2H:3H]

The weight fold (1024x1024 matvec) is done on host in fp64; the heavy part
(64*512 dot products of length 1024 + softmax) runs on 8 NeuronCores,
data-parallel over batch (8 batches per core).

Kernel strategy (v3):
  * enc ships as fp16 (half the HBM traffic of fp32; the kernel is
    DMA-bound and the 2^-11 input rounding moves scores by ~3e-3 -- two
    orders inside the 2e-2 gate).  Host pre-transposes each batch to
    [H, S] so the contraction dim lands on SBUF partitions.
  * ONE 1MiB DMA per batch: descriptor generation (HWDGE) is a serial
    ~650ns/DMA resource, so few big DMAs keep the stream transfer-bound
    (2913ns/batch at 360B/ns).  The batch-0 DMA is issued first so no
    other descriptor-gen delays the stream start.
  * The dots run on the PE array: for each (batch, s-chunk, h-chunk) the
    128x128 enc chunk is the *stationary* operand and the matching 128-row
    slice of w_enc is a single moving column, accumulating into a
    [128, 1] PSUM column over the 8 h-chunks.  Output-free-size-1 matmuls
    leave the PE essentially idle (and immune to p-state), so the DMA
    stream is the only real cost.
  * Softmax tail: one PSUM->SBUF copy of the [128, 32] scores, 4 PE
    transposes into a single batch-major [8, 512] PSUM tile, ONE exp on
    ACT with free-dim sum accumulation, then a fused divide on DVE and
    one small output DMA.
  * use_scatter=True switches the output to a SWDGE scatter-add whose
    descriptors are prepared during the stream and fired by trigger_dma
    (saves ~1.1us of HWDGE gen + DGE latency in the cost model).  It is
    numerically correct in CoreSim but produced garbage on real hardware
    under the bass2jax/axon execution path, so it is OFF by default.
"""

import sys
import types

import numpy as np
import concourse.bacc as bacc
import concourse.bass as bass
import concourse.mybir as mybir
import concourse.tile as tile
from concourse.bass_utils import run_bass_kernel_spmd

# run_bass_kernel_spmd(trace=True) (e.g. via BASS_TRACE=1 in the env)
# imports antenv.axon_hooks, which does not exist in this container. Register
# a stub returning "no hook" so tracing degrades gracefully instead of
# raising ModuleNotFoundError.
try:
    import antenv.axon_hooks  # noqa: F401
except ImportError:
    try:
        import antenv

        _stub = types.ModuleType("antenv.axon_hooks")
        _stub.get_axon_ntff_profile_hook = lambda: None  # type: ignore[attr-defined]
        sys.modules["antenv.axon_hooks"] = _stub
        antenv.axon_hooks = _stub
    except ImportError:
        pass

N_CORES = 8
B, S, H = 64, 512, 1024
P = 128             # SBUF partitions
BPC = B // N_CORES  # batches per core = 8
JT = S // P         # s-chunks per batch = 4
HC = H // P         # h-chunks = 8

F32 = mybir.dt.float32
F16 = mybir.dt.float16
I16 = mybir.dt.int16

_compiled_nc = None
LAST_RESULTS = None  # BassKernelResults of the most recent run (for profiling)


def _build_nc(dma_only=False, compute_only=False, use_scatter=False):
    """Per-core kernel: probs[BPC, S] = softmax_s(enc[BPC, S, H] @ w_enc).

    dma_only / compute_only build crippled variants for cost attribution.
    """
    # Bacc (not raw Bass): its compile() legalizes multi-wait instructions
    # into EventSemaphore waits (TRN2 allows only 1 sync wait per inst).
    nc = bacc.Bacc("TRN2", target_bir_lowering=False, debug=False)

    # enc arrives pre-transposed per batch: [BPC, HC, P, S] fp16 where
    # enc_t[b, c, p, s] = enc[b, s, c*128+p].
    enc_d = nc.dram_tensor("enc_in", [BPC, HC, P, S], F16, kind="ExternalInput")
    # w_col[p, c] = w_enc[c*128 + p]
    w_d = nc.dram_tensor("w_in", [P, HC], F16, kind="ExternalInput")
    # scatter indices: row i -> output row i for i < BPC, -1 (ignored) after
    sidx_d = nc.dram_tensor("sidx_in", [P, 1], I16, kind="ExternalInput")
    out_d = nc.dram_tensor("probs_out", [BPC, S], F32, kind="ExternalOutput")

    enc = enc_d.ap()

    with tile.TileContext(nc) as tc:
        with (
            tc.tile_pool(name="const", bufs=1) as constp,
            tc.tile_pool(name="ebuf", bufs=BPC) as ebufp,
            tc.tile_pool(name="small", bufs=1) as smallp,
            tc.tile_pool(name="psum", bufs=1, space="PSUM") as psump,
        ):
            # Batch-0 enc DMA first: nothing delays the start of the
            # transfer stream (every other DMA's descriptor-gen then hides
            # behind a running transfer).
            ets = []
            for b in range(BPC):
                ets.append(ebufp.tile([P, HC, S], F16, name="et", tag="et"))

            def enc_dma(b):
                if compute_only:
                    nc.sync.dma_start(ets[b][0:1, 0:1, 0:1], enc[b, 0, 0:1, 0:1])
                else:
                    nc.sync.dma_start(
                        ets[b][:], enc[b].rearrange("c p s -> p c s")
                    )

            enc_dma(0)

            # w next: tiny (2KiB), gates the first matmul.
            w_col = constp.tile([P, HC], F16, name="w_col")
            nc.sync.dma_start(w_col[:], w_d.ap())

            enc_dma(1)

            # Output plumbing, all off the critical path:
            #  - zeros DMA'd over the output region (the scatter ADDs),
            #  - scatter indices,
            #  - descriptor PREP for the output scatter (SWDGE, Pool).
            prob = smallp.tile([P, S], F32, name="prob")
            nc.gpsimd.memset(prob[:], 0.0)
            if use_scatter:
                ztile = smallp.tile([BPC, S], F32, name="ztile")
                nc.gpsimd.memset(ztile[:], 0.0)
                nc.sync.dma_start(out_d.ap(), ztile[:])
                sidx = constp.tile([P, 1], I16, name="sidx")
                nc.sync.dma_start(sidx[:], sidx_d.ap())

                # prob is a full [128, S] tile (scatter shape contract);
                # rows >= BPC carry zeros (memset above) re-added to rows
                # 0-7 by tokens 8-15.
                # Completion sem must be the Tile-managed DMASW0 lane sem:
                # the end-of-kernel drain waits on it, and with prepare_only
                # the descriptor (fired by trigger_dma) increments it.
                nc.gpsimd.dma_scatter_add(
                    out_d.ap(),
                    prob[:].unsqueeze(1),  # [128,1,S]: 128*1 == roundup(16,128)
                    sidx[:],
                    16,
                    16,
                    S,
                    prepare_only=True,
                    sem=tc.sems.swdge_block()[0],
                )

            for b2 in range(2, BPC):
                enc_dma(b2)

            # identity for the PE transposes, built on-device (gpsimd is
            # otherwise idle): ones everywhere, keep only where p - f == 0.
            ones_id = constp.tile([P, P], F32, name="ones_id")
            nc.gpsimd.memset(ones_id[:], 1.0)
            id_t = constp.tile([P, P], F32, name="id_t")
            nc.gpsimd.affine_select(
                out=id_t[:],
                in_=ones_id[:],
                pattern=[[-1, P]],
                compare_op=mybir.AluOpType.is_equal,
                fill=0.0,
                channel_multiplier=1,
            )

            if dma_only:
                nc.vector.tensor_copy(prob[0:BPC, :], ets[0][0:BPC, 0, 0:S])
                if use_scatter:
                    nc.gpsimd.trigger_dma(count=None)
                else:
                    nc.sync.dma_start(out_d.ap(), prob[0:BPC, :])
            else:
                # scores ps[p, j*BPC + b] = enc[b, j*128+p, :] . w_enc,
                # accumulated over the 8 h-chunks on the PE array.
                ps = psump.tile([P, JT * BPC], F32, name="ps")
                for b in range(BPC):
                    for j in range(JT):
                        ci = j * BPC + b
                        for c in range(HC):
                            nc.tensor.matmul(
                                ps[:, ci : ci + 1],
                                ets[b][:, c, j * P : (j + 1) * P],
                                w_col[:, c : c + 1],
                                start=(c == 0),
                                stop=(c == HC - 1),
                            )

                scores = smallp.tile([P, JT * BPC], F32, name="scores")
                nc.vector.tensor_copy(scores[:], ps[:])

                # transpose scores -> batch-major [8, 512] in ONE PSUM tile
                # (one bank): a single exp covers all of it afterwards.
                psumT = psump.tile([BPC, S], F32, name="psumT")
                for j in range(JT):
                    nc.tensor.transpose(
                        psumT[:, j * P : (j + 1) * P],
                        scores[:, j * BPC : (j + 1) * BPC],
                        id_t[:],
                    )

                # softmax over the free dim (fully local per batch row).
                # No max-subtraction: scores are bounded well inside fp32
                # exp range (|score| < ~60) and softmax is shift-invariant.
                # ONE exp with free-dim sum accumulation -> sums in a
                # single ACT op.
                expt = smallp.tile([BPC, S], F32, name="expt")
                sums = smallp.tile([BPC, 1], F32, name="sums")
                nc.scalar.activation(
                    out=expt[:],
                    in_=psumT[:],
                    func=mybir.ActivationFunctionType.Exp,
                    bias=0.0,
                    scale=1.0,
                    accum_out=sums[:],
                )
                binv = smallp.tile([BPC, 1], F32, name="binv")
                nc.vector.reciprocal(binv[:], sums[:])
                nc.vector.tensor_scalar_mul(prob[0:BPC, :], expt[:], binv[:])

                if use_scatter:
                    # fire the pre-generated output descriptors (SWDGE): the
                    # trigger carries the data dep on prob, the prep did not.
                    nc.gpsimd.trigger_dma(count=None)
                    # consume the scatter's completion sem before the
                    # end-of-scope sem-range clear (race detector).
                    nc.gpsimd.wait_ge(tc.sems.swdge_block()[0], 16)
                else:
                    nc.sync.dma_start(out_d.ap(), prob[0:BPC, :])

    nc.finalize()  # Bacc: runs compile() (wait legalization, reg alloc, ...)
    return nc


def kernel(hidden, encoder_outputs, W, b, v):
    global _compiled_nc, LAST_RESULTS

    # Fold the linear layer on host (fp64 for accuracy): only the
    # encoder-input slice of W survives the softmax. Force numpy so the fold
    # never runs through a jax device backend.
    W = np.asarray(W)
    v = np.asarray(v)
    w_enc = (v.astype(np.float64) @ W[:, 2 * H :].astype(np.float64)).astype(
        np.float32
    )
    # w_col[p, c] = w_enc[c*128 + p]
    w_col = np.ascontiguousarray(w_enc.reshape(HC, P).T).astype(np.float16)
    # enc_t[b, c, p, s] = enc[b, s, c*128+p], fp16
    enc = np.asarray(encoder_outputs).astype(np.float16)
    enc_t = np.ascontiguousarray(
        enc.reshape(B, S, HC, P).transpose(0, 2, 3, 1)
    )
    # 16 scatter tokens: tokens 0-7 carry the probs; tokens 8-15 re-target
    # rows 0-7 but read prob rows 8-15, which are memset to zero on device,
    # so they add 0.  (All-valid indices keep the DMA completion semaphore
    # at its expected count of 16.)
    sidx = np.full((128, 1), -1, dtype=np.int16)
    sidx[:BPC, 0] = np.arange(BPC, dtype=np.int16)
    sidx[BPC:16, 0] = np.arange(BPC, dtype=np.int16)

    if _compiled_nc is None:
        _compiled_nc = _build_nc()

    in_maps = [
        {
            "enc_in": enc_t[c * BPC : (c + 1) * BPC],
            "w_in": w_col,
            "sidx_in": sidx,
        }
        for c in range(N_CORES)
    ]
    # The axon/PJRT path sporadically throws a transient INTERNAL error at
    # compile time; a retry has always succeeded.
    last_err = None
    for _attempt in range(3):
        try:
            LAST_RESULTS = run_bass_kernel_spmd(
                _compiled_nc, in_maps, core_ids=list(range(N_CORES))
            )
            break
        except Exception as e:  # noqa: BLE001
            last_err = e
    else:
        raise last_err
    out = np.concatenate([r["probs_out"] for r in LAST_RESULTS.results], axis=0)
    return out.astype(np.float32)


# revision 32
# speedup vs baseline: 1.0098x; 1.0098x over previous
"""Trainium2 Bass kernel for nn_Attn_58669253263845 (sparse_attention).

Reference computation:
    hidden2 = concat(hidden[0], hidden[1])                 # [B, 2H]
    attn_input = concat(bcast(hidden2), encoder_outputs)   # [B, S, 3H]
    energy = attn_input @ W.T + b                          # [B, S, H]
    scores = energy @ v                                    # [B, S]
    out = softmax(scores, axis=S)

Everything before the softmax is linear, so
    scores[b,s] = attn_input[b,s,:] . (v @ W) + v.b
                = hidden2[b,:] . w_hid + enc[b,s,:] . w_enc + v.b
The hidden/bias terms are constant per batch row and cancel in the softmax
over S.  Hence:
    out = softmax_s(enc[b,s,:] . w_enc),  w_enc = v @ W[:, 2H:3H]

The weight fold (1024x1024 matvec) is done on host in fp64; the heavy part
(64*512 dot products of length 1024 + softmax) runs on 8 NeuronCores,
data-parallel over batch (8 batches per core).

Kernel strategy (v3):
  * enc ships as fp16 (half the HBM traffic of fp32; the kernel is
    DMA-bound and the 2^-11 input rounding moves scores by ~3e-3 -- two
    orders inside the 2e-2 gate).  Host pre-transposes each batch to
    [H, S] so the contraction dim lands on SBUF partitions.
  * ONE 1MiB DMA per batch: descriptor generation (HWDGE) is a serial
    ~650ns/DMA resource, so few big DMAs keep the stream transfer-bound
    (2913ns/batch at 360B/ns).  The batch-0 DMA is issued first so no
    other descriptor-gen delays the stream start.
  * The dots run on the PE array: for each (batch, s-chunk, h-chunk) the
    128x128 enc chunk is the *stationary* operand and the matching 128-row
    slice of w_enc is a single moving column, accumulating into a
    [128, 1] PSUM column over the 8 h-chunks.  Output-free-size-1 matmuls
    leave the PE essentially idle (and immune to p-state), so the DMA
    stream is the only real cost.
  * Softmax tail: one PSUM->SBUF copy of the [128, 32] scores, 4 PE
    transposes into a single batch-major [8, 512] PSUM tile, ONE exp on
    ACT with free-dim sum accumulation, then a fused divide on DVE and
    one small output DMA.
  * use_scatter=True switches the output to a SWDGE scatter-add whose
    descriptors are prepared during the stream and fired by trigger_dma
    (saves ~1.1us of HWDGE gen + DGE latency in the cost model).  It is
    numerically correct in CoreSim but produced garbage on real hardware
    under the bass2jax/axon execution path, so it is OFF by default.
"""

import sys
import types

import numpy as np
import concourse.bacc as bacc
import concourse.bass as bass
import concourse.mybir as mybir
import concourse.tile as tile
from concourse.bass_utils import run_bass_kernel_spmd

# run_bass_kernel_spmd(trace=True) (e.g. via BASS_TRACE=1 in the env)
# imports antenv.axon_hooks, which does not exist in this container. Register
# a stub returning "no hook" so tracing degrades gracefully instead of
# raising ModuleNotFoundError.
try:
    import antenv.axon_hooks  # noqa: F401
except ImportError:
    try:
        import antenv

        _stub = types.ModuleType("antenv.axon_hooks")
        _stub.get_axon_ntff_profile_hook = lambda: None  # type: ignore[attr-defined]
        sys.modules["antenv.axon_hooks"] = _stub
        antenv.axon_hooks = _stub
    except ImportError:
        pass

N_CORES = 8
B, S, H = 64, 512, 1024
P = 128             # SBUF partitions
BPC = B // N_CORES  # batches per core = 8
JT = S // P         # s-chunks per batch = 4
HC = H // P         # h-chunks = 8

F32 = mybir.dt.float32
F16 = mybir.dt.float16
I16 = mybir.dt.int16

_compiled_nc = None
LAST_RESULTS = None  # BassKernelResults of the most recent run (for profiling)


def _build_nc(dma_only=False, compute_only=False, use_scatter=False):
    """Per-core kernel: probs[BPC, S] = softmax_s(enc[BPC, S, H] @ w_enc).

    dma_only / compute_only build crippled variants for cost attribution.
    """
    # Bacc (not raw Bass): its compile() legalizes multi-wait instructions
    # into EventSemaphore waits (TRN2 allows only 1 sync wait per inst).
    nc = bacc.Bacc("TRN2", target_bir_lowering=False, debug=False)

    # enc arrives pre-transposed per batch: [BPC, HC, P, S] fp16 where
    # enc_t[b, c, p, s] = enc[b, s, c*128+p].
    enc_d = nc.dram_tensor("enc_in", [BPC, HC, P, S], F16, kind="ExternalInput")
    # w_col[p, c] = w_enc[c*128 + p]
    w_d = nc.dram_tensor("w_in", [P, HC], F16, kind="ExternalInput")
    # scatter indices: row i -> output row i for i < BPC, -1 (ignored) after
    sidx_d = nc.dram_tensor("sidx_in", [P, 1], I16, kind="ExternalInput")
    out_d = nc.dram_tensor("probs_out", [BPC, S], F32, kind="ExternalOutput")

    enc = enc_d.ap()

    with tile.TileContext(nc) as tc:
        with (
            tc.tile_pool(name="const", bufs=1) as constp,
            tc.tile_pool(name="ebuf", bufs=BPC + 1) as ebufp,
            tc.tile_pool(name="small", bufs=1) as smallp,
            tc.tile_pool(name="psum", bufs=1, space="PSUM") as psump,
        ):
            # Batch-0 enc DMA first: nothing delays the start of the
            # transfer stream (every other DMA's descriptor-gen then hides
            # behind a running transfer).
            ets = []
            for b in range(BPC - 1):
                ets.append(ebufp.tile([P, HC, S], F16, name="et", tag="et"))

            def enc_dma(b):
                if compute_only:
                    nc.sync.dma_start(ets[b][0:1, 0:1, 0:1], enc[b, 0, 0:1, 0:1])
                else:
                    nc.sync.dma_start(
                        ets[b][:], enc[b].rearrange("c p s -> p c s")
                    )

            enc_dma(0)

            # w next: tiny (2KiB), gates the first matmul.
            w_col = constp.tile([P, HC], F16, name="w_col")
            nc.sync.dma_start(w_col[:], w_d.ap())

            enc_dma(1)

            # Output plumbing, all off the critical path:
            #  - zeros DMA'd over the output region (the scatter ADDs),
            #  - scatter indices,
            #  - descriptor PREP for the output scatter (SWDGE, Pool).
            prob = smallp.tile([P, S], F32, name="prob")
            nc.gpsimd.memset(prob[:], 0.0)
            if use_scatter:
                ztile = smallp.tile([BPC, S], F32, name="ztile")
                nc.gpsimd.memset(ztile[:], 0.0)
                nc.sync.dma_start(out_d.ap(), ztile[:])
                sidx = constp.tile([P, 1], I16, name="sidx")
                nc.sync.dma_start(sidx[:], sidx_d.ap())

                # prob is a full [128, S] tile (scatter shape contract);
                # rows >= BPC carry zeros (memset above) re-added to rows
                # 0-7 by tokens 8-15.
                # Completion sem must be the Tile-managed DMASW0 lane sem:
                # the end-of-kernel drain waits on it, and with prepare_only
                # the descriptor (fired by trigger_dma) increments it.
                nc.gpsimd.dma_scatter_add(
                    out_d.ap(),
                    prob[:].unsqueeze(1),  # [128,1,S]: 128*1 == roundup(16,128)
                    sidx[:],
                    16,
                    16,
                    S,
                    prepare_only=True,
                    sem=tc.sems.swdge_block()[0],
                )

            for b2 in range(2, BPC - 1):
                enc_dma(b2)

            # The LAST batch arrives as two s-halves (s 0:256 and 256:512,
            # i.e. j-chunks {0,1} and {2,3}): the j0/j1 half of the score
            # pipeline (matmuls, copy, transposes, exp) then overlaps the
            # second half's 1456ns transfer, shortening the exposed tail.
            SH = S // 2
            et7 = [
                ebufp.tile([P, HC, SH], F16, name="et7", tag=f"et7{h}")
                for h in range(2)
            ]
            for h in range(2):
                if compute_only:
                    nc.sync.dma_start(
                        et7[h][0:1, 0:1, 0:1], enc[BPC - 1, 0, 0:1, 0:1]
                    )
                else:
                    nc.sync.dma_start(
                        et7[h][:],
                        enc[BPC - 1].rearrange("c p s -> p c s")[
                            :, :, h * SH : (h + 1) * SH
                        ],
                    )

            # identity for the PE transposes, built on-device (gpsimd is
            # otherwise idle): ones everywhere, keep only where p - f == 0.
            ones_id = constp.tile([P, P], F32, name="ones_id")
            nc.gpsimd.memset(ones_id[:], 1.0)
            id_t = constp.tile([P, P], F32, name="id_t")
            nc.gpsimd.affine_select(
                out=id_t[:],
                in_=ones_id[:],
                pattern=[[-1, P]],
                compare_op=mybir.AluOpType.is_equal,
                fill=0.0,
                channel_multiplier=1,
            )

            if dma_only:
                nc.vector.tensor_copy(prob[0:BPC, :], ets[0][0:BPC, 0, 0:S])
                if use_scatter:
                    nc.gpsimd.trigger_dma(count=None)
                else:
                    nc.sync.dma_start(out_d.ap(), prob[0:BPC, :])
            else:
                # scores ps[p, j*BPC + b] = enc[b, j*128+p, :] . w_enc,
                # accumulated over the 8 h-chunks on the PE array.
                def lhs(b, j, c):
                    if b < BPC - 1:
                        return ets[b][:, c, j * P : (j + 1) * P]
                    return et7[j // 2][:, c, (j % 2) * P : (j % 2 + 1) * P]

                # Scores for j-chunks {0,1} and {2,3} accumulate in
                # SEPARATE full-bank PSUM tiles: PSUM dependency tracking is
                # bank-granular, so sharing a bank would gate the stage-1
                # copy on the very last matmul and kill the overlap.
                ps = [
                    psump.tile([P, S], F32, name=f"ps{h}") for h in range(2)
                ]
                for b in range(BPC):
                    for j in range(JT):
                        ci = (j % 2) * BPC + b
                        for c in range(HC):
                            nc.tensor.matmul(
                                ps[j // 2][:, ci : ci + 1],
                                lhs(b, j, c),
                                w_col[:, c : c + 1],
                                start=(c == 0),
                                stop=(c == HC - 1),
                            )

                # Two-stage tail, one stage per s-half (j-chunks {0,1} and
                # {2,3}).  Each half gets its own scores copy, transposes
                # into its OWN full-bank PSUM tile (bank-granular dep
                # tracking: sharing one bank would chain exp1 behind T3),
                # and its own exp with row-sum accumulation.  Stage 1 is
                # gated only by the first half-DMA of the last batch, so it
                # runs under the second half's transfer.
                scores = smallp.tile([P, JT * BPC], F32, name="scores")
                expt = smallp.tile([BPC, S], F32, name="expt")
                sums2 = smallp.tile([BPC, 2], F32, name="sums2")
                psumT = [
                    psump.tile([BPC, S], F32, name=f"psumT{h}") for h in range(2)
                ]
                for h in range(2):
                    cols = slice(h * 2 * BPC, (h + 1) * 2 * BPC)
                    nc.vector.tensor_copy(
                        scores[:, cols], ps[h][:, 0 : 2 * BPC]
                    )
                    for j in (2 * h, 2 * h + 1):
                        nc.tensor.transpose(
                            psumT[h][:, (j % 2) * P : (j % 2 + 1) * P],
                            scores[:, j * BPC : (j + 1) * BPC],
                            id_t[:],
                        )
                    nc.scalar.activation(
                        out=expt[:, h * SH : (h + 1) * SH],
                        in_=psumT[h][:, 0 : 2 * P],
                        func=mybir.ActivationFunctionType.Exp,
                        bias=0.0,
                        scale=1.0,
                        accum_out=sums2[:, h : h + 1],
                    )

                sums = smallp.tile([BPC, 1], F32, name="sums")
                nc.vector.tensor_reduce(
                    out=sums[:],
                    in_=sums2[:],
                    axis=mybir.AxisListType.X,
                    op=mybir.AluOpType.add,
                )
                binv = smallp.tile([BPC, 1], F32, name="binv")
                nc.vector.reciprocal(binv[:], sums[:])
                nc.vector.tensor_scalar_mul(prob[0:BPC, :], expt[:], binv[:])

                if use_scatter:
                    # fire the pre-generated output descriptors (SWDGE): the
                    # trigger carries the data dep on prob, the prep did not.
                    nc.gpsimd.trigger_dma(count=None)
                    # consume the scatter's completion sem before the
                    # end-of-scope sem-range clear (race detector).
                    nc.gpsimd.wait_ge(tc.sems.swdge_block()[0], 16)
                else:
                    nc.sync.dma_start(out_d.ap(), prob[0:BPC, :])

    nc.finalize()  # Bacc: runs compile() (wait legalization, reg alloc, ...)
    return nc


def kernel(hidden, encoder_outputs, W, b, v):
    global _compiled_nc, LAST_RESULTS

    # Fold the linear layer on host (fp64 for accuracy): only the
    # encoder-input slice of W survives the softmax. Force numpy so the fold
    # never runs through a jax device backend.
    W = np.asarray(W)
    v = np.asarray(v)
    w_enc = (v.astype(np.float64) @ W[:, 2 * H :].astype(np.float64)).astype(
        np.float32
    )
    # w_col[p, c] = w_enc[c*128 + p]
    w_col = np.ascontiguousarray(w_enc.reshape(HC, P).T).astype(np.float16)
    # enc_t[b, c, p, s] = enc[b, s, c*128+p], fp16
    enc = np.asarray(encoder_outputs).astype(np.float16)
    enc_t = np.ascontiguousarray(
        enc.reshape(B, S, HC, P).transpose(0, 2, 3, 1)
    )
    # 16 scatter tokens: tokens 0-7 carry the probs; tokens 8-15 re-target
    # rows 0-7 but read prob rows 8-15, which are memset to zero on device,
    # so they add 0.  (All-valid indices keep the DMA completion semaphore
    # at its expected count of 16.)
    sidx = np.full((128, 1), -1, dtype=np.int16)
    sidx[:BPC, 0] = np.arange(BPC, dtype=np.int16)
    sidx[BPC:16, 0] = np.arange(BPC, dtype=np.int16)

    if _compiled_nc is None:
        _compiled_nc = _build_nc()

    in_maps = [
        {
            "enc_in": enc_t[c * BPC : (c + 1) * BPC],
            "w_in": w_col,
            "sidx_in": sidx,
        }
        for c in range(N_CORES)
    ]
    # The axon/PJRT path sporadically throws a transient INTERNAL error at
    # compile time; a retry has always succeeded.
    last_err = None
    for _attempt in range(3):
        try:
            LAST_RESULTS = run_bass_kernel_spmd(
                _compiled_nc, in_maps, core_ids=list(range(N_CORES))
            )
            break
        except Exception as e:  # noqa: BLE001
            last_err = e
    else:
        raise last_err
    out = np.concatenate([r["probs_out"] for r in LAST_RESULTS.results], axis=0)
    return out.astype(np.float32)


# revision 35
# speedup vs baseline: 1.0103x; 1.0005x over previous
"""Trainium2 Bass kernel for nn_Attn_58669253263845 (sparse_attention).

Reference computation:
    hidden2 = concat(hidden[0], hidden[1])                 # [B, 2H]
    attn_input = concat(bcast(hidden2), encoder_outputs)   # [B, S, 3H]
    energy = attn_input @ W.T + b                          # [B, S, H]
    scores = energy @ v                                    # [B, S]
    out = softmax(scores, axis=S)

Everything before the softmax is linear, so
    scores[b,s] = attn_input[b,s,:] . (v @ W) + v.b
                = hidden2[b,:] . w_hid + enc[b,s,:] . w_enc + v.b
The hidden/bias terms are constant per batch row and cancel in the softmax
over S.  Hence:
    out = softmax_s(enc[b,s,:] . w_enc),  w_enc = v @ W[:, 2H:3H]

The weight fold (1024x1024 matvec) is done on host in fp64; the heavy part
(64*512 dot products of length 1024 + softmax) runs on 8 NeuronCores,
data-parallel over batch (8 batches per core).

Kernel strategy (v3):
  * enc ships as fp16 (half the HBM traffic of fp32; the kernel is
    DMA-bound and the 2^-11 input rounding moves scores by ~3e-3 -- two
    orders inside the 2e-2 gate).  Host pre-transposes each batch to
    [H, S] so the contraction dim lands on SBUF partitions.
  * ONE 1MiB DMA per batch: descriptor generation (HWDGE) is a serial
    ~650ns/DMA resource, so few big DMAs keep the stream transfer-bound
    (2913ns/batch at 360B/ns).  The batch-0 DMA is issued first so no
    other descriptor-gen delays the stream start.
  * The dots run on the PE array: for each (batch, s-chunk, h-chunk) the
    128x128 enc chunk is the *stationary* operand and the matching 128-row
    slice of w_enc is a single moving column, accumulating into a
    [128, 1] PSUM column over the 8 h-chunks.  Output-free-size-1 matmuls
    leave the PE essentially idle (and immune to p-state), so the DMA
    stream is the only real cost.
  * Two-stage softmax tail: the last batch arrives as two s-half DMAs,
    and the scores for j-chunks {0,1} accumulate in their OWN full-bank
    PSUM tile (PSUM dep tracking is bank-granular), so stage 1 (copy +
    2 transposes + exp[8,256] with row-sum accumulation) runs UNDER the
    second half's 1456ns transfer.  Only stage 2 + reduce + reciprocal +
    scale + one small output DMA are exposed after the final byte.
  * use_scatter=True switches the output to a SWDGE scatter-add whose
    descriptors are prepared during the stream and fired by trigger_dma
    (saves ~1.1us of HWDGE gen + DGE latency in the cost model).  It is
    numerically correct in CoreSim but produced garbage on real hardware
    under the bass2jax/axon execution path, so it is OFF by default.
"""

import sys
import types

import numpy as np
import concourse.bacc as bacc
import concourse.bass as bass
import concourse.mybir as mybir
import concourse.tile as tile
from concourse.bass_utils import run_bass_kernel_spmd

# run_bass_kernel_spmd(trace=True) (e.g. via BASS_TRACE=1 in the env)
# imports antenv.axon_hooks, which does not exist in this container. Register
# a stub returning "no hook" so tracing degrades gracefully instead of
# raising ModuleNotFoundError.
try:
    import antenv.axon_hooks  # noqa: F401
except ImportError:
    try:
        import antenv

        _stub = types.ModuleType("antenv.axon_hooks")
        _stub.get_axon_ntff_profile_hook = lambda: None  # type: ignore[attr-defined]
        sys.modules["antenv.axon_hooks"] = _stub
        antenv.axon_hooks = _stub
    except ImportError:
        pass

N_CORES = 8
B, S, H = 64, 512, 1024
P = 128             # SBUF partitions
BPC = B // N_CORES  # batches per core = 8
JT = S // P         # s-chunks per batch = 4
HC = H // P         # h-chunks = 8

F32 = mybir.dt.float32
F16 = mybir.dt.float16
I16 = mybir.dt.int16

_compiled_nc = None
LAST_RESULTS = None  # BassKernelResults of the most recent run (for profiling)


def _build_nc(dma_only=False, compute_only=False, use_scatter=False):
    """Per-core kernel: probs[BPC, S] = softmax_s(enc[BPC, S, H] @ w_enc).

    dma_only / compute_only build crippled variants for cost attribution.
    """
    # Bacc (not raw Bass): its compile() legalizes multi-wait instructions
    # into EventSemaphore waits (TRN2 allows only 1 sync wait per inst).
    nc = bacc.Bacc("TRN2", target_bir_lowering=False, debug=False)

    # enc arrives pre-transposed per batch: [BPC, HC, P, S] fp16 where
    # enc_t[b, c, p, s] = enc[b, s, c*128+p].
    enc_d = nc.dram_tensor("enc_in", [BPC, HC, P, S], F16, kind="ExternalInput")
    # w_col[p, c] = w_enc[c*128 + p]
    w_d = nc.dram_tensor("w_in", [P, HC], F16, kind="ExternalInput")
    # scatter indices: row i -> output row i for i < BPC, -1 (ignored) after
    sidx_d = nc.dram_tensor("sidx_in", [P, 1], I16, kind="ExternalInput")
    out_d = nc.dram_tensor("probs_out", [BPC, S], F32, kind="ExternalOutput")

    enc = enc_d.ap()

    with tile.TileContext(nc) as tc:
        with (
            tc.tile_pool(name="const", bufs=1) as constp,
            tc.tile_pool(name="ebuf", bufs=BPC + 1) as ebufp,
            tc.tile_pool(name="small", bufs=1) as smallp,
            tc.tile_pool(name="psum", bufs=1, space="PSUM") as psump,
        ):
            # Batch-0 enc DMA first: nothing delays the start of the
            # transfer stream (every other DMA's descriptor-gen then hides
            # behind a running transfer).
            ets = []
            for b in range(BPC - 1):
                ets.append(ebufp.tile([P, HC, S], F16, name="et", tag="et"))

            def enc_dma(b):
                if compute_only:
                    nc.sync.dma_start(ets[b][0:1, 0:1, 0:1], enc[b, 0, 0:1, 0:1])
                else:
                    nc.sync.dma_start(
                        ets[b][:], enc[b].rearrange("c p s -> p c s")
                    )

            enc_dma(0)

            # w next: tiny (2KiB), gates the first matmul.
            w_col = constp.tile([P, HC], F16, name="w_col")
            nc.sync.dma_start(w_col[:], w_d.ap())

            enc_dma(1)

            # Output plumbing, all off the critical path:
            #  - zeros DMA'd over the output region (the scatter ADDs),
            #  - scatter indices,
            #  - descriptor PREP for the output scatter (SWDGE, Pool).
            prob = smallp.tile([P, S], F32, name="prob")
            nc.gpsimd.memset(prob[:], 0.0)
            if use_scatter:
                ztile = smallp.tile([BPC, S], F32, name="ztile")
                nc.gpsimd.memset(ztile[:], 0.0)
                nc.sync.dma_start(out_d.ap(), ztile[:])
                sidx = constp.tile([P, 1], I16, name="sidx")
                nc.sync.dma_start(sidx[:], sidx_d.ap())

                # prob is a full [128, S] tile (scatter shape contract);
                # rows >= BPC carry zeros (memset above) re-added to rows
                # 0-7 by tokens 8-15.
                # Completion sem must be the Tile-managed DMASW0 lane sem:
                # the end-of-kernel drain waits on it, and with prepare_only
                # the descriptor (fired by trigger_dma) increments it.
                nc.gpsimd.dma_scatter_add(
                    out_d.ap(),
                    prob[:].unsqueeze(1),  # [128,1,S]: 128*1 == roundup(16,128)
                    sidx[:],
                    16,
                    16,
                    S,
                    prepare_only=True,
                    sem=tc.sems.swdge_block()[0],
                )

            for b2 in range(2, BPC - 1):
                enc_dma(b2)

            # The LAST batch arrives as two s-halves (s 0:256 and 256:512,
            # i.e. j-chunks {0,1} and {2,3}): the j0/j1 half of the score
            # pipeline (matmuls, copy, transposes, exp) then overlaps the
            # second half's 1456ns transfer, shortening the exposed tail.
            SH = S // 2
            et7 = [
                ebufp.tile([P, HC, SH], F16, name="et7", tag=f"et7{h}")
                for h in range(2)
            ]
            for h in range(2):
                if compute_only:
                    nc.sync.dma_start(
                        et7[h][0:1, 0:1, 0:1], enc[BPC - 1, 0, 0:1, 0:1]
                    )
                else:
                    nc.sync.dma_start(
                        et7[h][:],
                        enc[BPC - 1].rearrange("c p s -> p c s")[
                            :, :, h * SH : (h + 1) * SH
                        ],
                    )

            # identity for the PE transposes, built on-device (gpsimd is
            # otherwise idle): ones everywhere, keep only where p - f == 0.
            ones_id = constp.tile([P, P], F32, name="ones_id")
            nc.gpsimd.memset(ones_id[:], 1.0)
            id_t = constp.tile([P, P], F32, name="id_t")
            nc.gpsimd.affine_select(
                out=id_t[:],
                in_=ones_id[:],
                pattern=[[-1, P]],
                compare_op=mybir.AluOpType.is_equal,
                fill=0.0,
                channel_multiplier=1,
            )

            if dma_only:
                nc.vector.tensor_copy(prob[0:BPC, :], ets[0][0:BPC, 0, 0:S])
                if use_scatter:
                    nc.gpsimd.trigger_dma(count=None)
                else:
                    nc.sync.dma_start(out_d.ap(), prob[0:BPC, :])
            else:
                # scores ps[p, j*BPC + b] = enc[b, j*128+p, :] . w_enc,
                # accumulated over the 8 h-chunks on the PE array.
                def lhs(b, j, c):
                    if b < BPC - 1:
                        return ets[b][:, c, j * P : (j + 1) * P]
                    return et7[j // 2][:, c, (j % 2) * P : (j % 2 + 1) * P]

                # Scores for j-chunks {0,1} and {2,3} accumulate in
                # SEPARATE full-bank PSUM tiles: PSUM dependency tracking is
                # bank-granular, so sharing a bank would gate the stage-1
                # copy on the very last matmul and kill the overlap.
                ps = [
                    psump.tile([P, S], F32, name=f"ps{h}") for h in range(2)
                ]
                psumT = [
                    psump.tile([BPC, S], F32, name=f"psumT{h}") for h in range(2)
                ]
                # The last batch's j2/j3 score columns accumulate in their
                # OWN bank: the bulk stage-2 copy (reading ps[1]) is then
                # gated by batch 6's matmuls, not batch 7's, and runs under
                # the final half-transfer; only a tiny strided copy of these
                # two columns stays in the exposed tail.
                psb7 = psump.tile([P, 2], F32, name="psb7")

                def score_col(b, j):
                    if b == BPC - 1 and j >= 2:
                        return psb7[:, (j - 2) : (j - 1)]
                    return ps[j // 2][:, (j % 2) * BPC + b : (j % 2) * BPC + b + 1]

                for b in range(BPC):
                    for j in range(JT):
                        for c in range(HC):
                            nc.tensor.matmul(
                                score_col(b, j),
                                lhs(b, j, c),
                                w_col[:, c : c + 1],
                                start=(c == 0),
                                stop=(c == HC - 1),
                            )

                # Two-stage tail, one stage per s-half (j-chunks {0,1} and
                # {2,3}).  Each half gets its own scores copy, transposes
                # into its OWN full-bank PSUM tile (bank-granular dep
                # tracking: sharing one bank would chain exp1 behind T3),
                # and its own exp with row-sum accumulation.  Stage 1 is
                # gated only by the first half-DMA of the last batch, so it
                # runs under the second half's transfer.
                scores = smallp.tile([P, JT * BPC], F32, name="scores")
                expt = smallp.tile([BPC, S], F32, name="expt")
                sums2 = smallp.tile([BPC, 2], F32, name="sums2")
                for h in range(2):
                    if h == 0:
                        nc.vector.tensor_copy(
                            scores[:, 0 : 2 * BPC], ps[0][:, 0 : 2 * BPC]
                        )
                    else:
                        # bulk (batches 0-6, early) ...
                        nc.vector.tensor_copy(
                            scores[:, 16:23], ps[1][:, 0 : BPC - 1]
                        )
                        nc.vector.tensor_copy(
                            scores[:, 24:31], ps[1][:, BPC : 2 * BPC - 1]
                        )
                        # ... then batch 7's two columns (after the last mms)
                        nc.vector.tensor_copy(
                            scores[:, 2 * BPC + BPC - 1 : JT * BPC : BPC], psb7[:, 0:2]
                        )
                    for j in (2 * h, 2 * h + 1):
                        nc.tensor.transpose(
                            psumT[h][:, (j % 2) * P : (j % 2 + 1) * P],
                            scores[:, j * BPC : (j + 1) * BPC],
                            id_t[:],
                        )
                    nc.scalar.activation(
                        out=expt[:, h * SH : (h + 1) * SH],
                        in_=psumT[h][:, 0 : 2 * P],
                        func=mybir.ActivationFunctionType.Exp,
                        bias=0.0,
                        scale=1.0,
                        accum_out=sums2[:, h : h + 1],
                    )

                sums = smallp.tile([BPC, 1], F32, name="sums")
                nc.vector.tensor_reduce(
                    out=sums[:],
                    in_=sums2[:],
                    axis=mybir.AxisListType.X,
                    op=mybir.AluOpType.add,
                )
                binv = smallp.tile([BPC, 1], F32, name="binv")
                nc.vector.reciprocal(binv[:], sums[:])
                nc.vector.tensor_scalar_mul(prob[0:BPC, :], expt[:], binv[:])

                if use_scatter:
                    # fire the pre-generated output descriptors (SWDGE): the
                    # trigger carries the data dep on prob, the prep did not.
                    nc.gpsimd.trigger_dma(count=None)
                    # consume the scatter's completion sem before the
                    # end-of-scope sem-range clear (race detector).
                    nc.gpsimd.wait_ge(tc.sems.swdge_block()[0], 16)
                else:
                    nc.sync.dma_start(out_d.ap(), prob[0:BPC, :])

    nc.finalize()  # Bacc: runs compile() (wait legalization, reg alloc, ...)
    return nc


def kernel(hidden, encoder_outputs, W, b, v):
    global _compiled_nc, LAST_RESULTS

    # Fold the linear layer on host (fp64 for accuracy): only the
    # encoder-input slice of W survives the softmax. Force numpy so the fold
    # never runs through a jax device backend.
    W = np.asarray(W)
    v = np.asarray(v)
    w_enc = (v.astype(np.float64) @ W[:, 2 * H :].astype(np.float64)).astype(
        np.float32
    )
    # w_col[p, c] = w_enc[c*128 + p]
    w_col = np.ascontiguousarray(w_enc.reshape(HC, P).T).astype(np.float16)
    # enc_t[b, c, p, s] = enc[b, s, c*128+p], fp16
    enc = np.asarray(encoder_outputs).astype(np.float16)
    enc_t = np.ascontiguousarray(
        enc.reshape(B, S, HC, P).transpose(0, 2, 3, 1)
    )
    # 16 scatter tokens: tokens 0-7 carry the probs; tokens 8-15 re-target
    # rows 0-7 but read prob rows 8-15, which are memset to zero on device,
    # so they add 0.  (All-valid indices keep the DMA completion semaphore
    # at its expected count of 16.)
    sidx = np.full((128, 1), -1, dtype=np.int16)
    sidx[:BPC, 0] = np.arange(BPC, dtype=np.int16)
    sidx[BPC:16, 0] = np.arange(BPC, dtype=np.int16)

    if _compiled_nc is None:
        _compiled_nc = _build_nc()

    in_maps = [
        {
            "enc_in": enc_t[c * BPC : (c + 1) * BPC],
            "w_in": w_col,
            "sidx_in": sidx,
        }
        for c in range(N_CORES)
    ]
    # The axon/PJRT path sporadically throws a transient INTERNAL error at
    # compile time; a retry has always succeeded.
    last_err = None
    for _attempt in range(3):
        try:
            LAST_RESULTS = run_bass_kernel_spmd(
                _compiled_nc, in_maps, core_ids=list(range(N_CORES))
            )
            break
        except Exception as e:  # noqa: BLE001
            last_err = e
    else:
        raise last_err
    out = np.concatenate([r["probs_out"] for r in LAST_RESULTS.results], axis=0)
    return out.astype(np.float32)


# revision 36
# speedup vs baseline: 1.0168x; 1.0064x over previous
"""Trainium2 Bass kernel for nn_Attn_58669253263845 (sparse_attention).

Reference computation:
    hidden2 = concat(hidden[0], hidden[1])                 # [B, 2H]
    attn_input = concat(bcast(hidden2), encoder_outputs)   # [B, S, 3H]
    energy = attn_input @ W.T + b                          # [B, S, H]
    scores = energy @ v                                    # [B, S]
    out = softmax(scores, axis=S)

Everything before the softmax is linear, so
    scores[b,s] = attn_input[b,s,:] . (v @ W) + v.b
                = hidden2[b,:] . w_hid + enc[b,s,:] . w_enc + v.b
The hidden/bias terms are constant per batch row and cancel in the softmax
over S.  Hence:
    out = softmax_s(enc[b,s,:] . w_enc),  w_enc = v @ W[:, 2H:3H]

The weight fold (1024x1024 matvec) is done on host in fp64; the heavy part
(64*512 dot products of length 1024 + softmax) runs on 8 NeuronCores,
data-parallel over batch (8 batches per core).

Kernel strategy (v3):
  * enc ships as fp16 (half the HBM traffic of fp32; the kernel is
    DMA-bound and the 2^-11 input rounding moves scores by ~3e-3 -- two
    orders inside the 2e-2 gate).  Host pre-transposes each batch to
    [H, S] so the contraction dim lands on SBUF partitions.
  * ONE 1MiB DMA per batch: descriptor generation (HWDGE) is a serial
    ~650ns/DMA resource, so few big DMAs keep the stream transfer-bound
    (2913ns/batch at 360B/ns).  The batch-0 DMA is issued first so no
    other descriptor-gen delays the stream start.
  * The dots run on the PE array: for each (batch, s-chunk, h-chunk) the
    128x128 enc chunk is the *stationary* operand and the matching 128-row
    slice of w_enc is a single moving column, accumulating into a
    [128, 1] PSUM column over the 8 h-chunks.  Output-free-size-1 matmuls
    leave the PE essentially idle (and immune to p-state), so the DMA
    stream is the only real cost.
  * Two-stage softmax tail: the last batch arrives as two s-half DMAs,
    and the scores for j-chunks {0,1} accumulate in their OWN full-bank
    PSUM tile (PSUM dep tracking is bank-granular), so stage 1 (copy +
    2 transposes + exp[8,256] with row-sum accumulation) runs UNDER the
    second half's 1456ns transfer.  Only stage 2 + reduce + reciprocal +
    scale + one small output DMA are exposed after the final byte.
  * use_scatter=True switches the output to a SWDGE scatter-add whose
    descriptors are prepared during the stream and fired by trigger_dma
    (saves ~1.1us of HWDGE gen + DGE latency in the cost model).  It is
    numerically correct in CoreSim but produced garbage on real hardware
    under the bass2jax/axon execution path, so it is OFF by default.
"""

import sys
import types

import numpy as np
import concourse.bacc as bacc
import concourse.bass as bass
import concourse.mybir as mybir
import concourse.tile as tile
from concourse.bass_utils import run_bass_kernel_spmd

# run_bass_kernel_spmd(trace=True) (e.g. via BASS_TRACE=1 in the env)
# imports antenv.axon_hooks, which does not exist in this container. Register
# a stub returning "no hook" so tracing degrades gracefully instead of
# raising ModuleNotFoundError.
try:
    import antenv.axon_hooks  # noqa: F401
except ImportError:
    try:
        import antenv

        _stub = types.ModuleType("antenv.axon_hooks")
        _stub.get_axon_ntff_profile_hook = lambda: None  # type: ignore[attr-defined]
        sys.modules["antenv.axon_hooks"] = _stub
        antenv.axon_hooks = _stub
    except ImportError:
        pass

N_CORES = 8
B, S, H = 64, 512, 1024
P = 128             # SBUF partitions
BPC = B // N_CORES  # batches per core = 8
JT = S // P         # s-chunks per batch = 4
HC = H // P         # h-chunks = 8

F32 = mybir.dt.float32
F16 = mybir.dt.float16
I16 = mybir.dt.int16

_compiled_nc = None
LAST_RESULTS = None  # BassKernelResults of the most recent run (for profiling)


def _build_nc(dma_only=False, compute_only=False, use_scatter=False):
    """Per-core kernel: probs[BPC, S] = softmax_s(enc[BPC, S, H] @ w_enc).

    dma_only / compute_only build crippled variants for cost attribution.
    """
    # Bacc (not raw Bass): its compile() legalizes multi-wait instructions
    # into EventSemaphore waits (TRN2 allows only 1 sync wait per inst).
    nc = bacc.Bacc("TRN2", target_bir_lowering=False, debug=False)

    # enc arrives pre-transposed per batch: [BPC, HC, P, S] fp16 where
    # enc_t[b, c, p, s] = enc[b, s, c*128+p].
    enc_d = nc.dram_tensor("enc_in", [BPC, HC, P, S], F16, kind="ExternalInput")
    # w_col[p, c] = w_enc[c*128 + p]
    w_d = nc.dram_tensor("w_in", [P, HC], F16, kind="ExternalInput")
    # scatter indices: row i -> output row i for i < BPC, -1 (ignored) after
    sidx_d = nc.dram_tensor("sidx_in", [P, 1], I16, kind="ExternalInput")
    out_d = nc.dram_tensor("probs_out", [BPC, S], F32, kind="ExternalOutput")

    enc = enc_d.ap()

    with tile.TileContext(nc) as tc:
        with (
            tc.tile_pool(name="const", bufs=1) as constp,
            tc.tile_pool(name="ebuf", bufs=BPC + 1) as ebufp,
            tc.tile_pool(name="small", bufs=1) as smallp,
            tc.tile_pool(name="psum", bufs=1, space="PSUM") as psump,
            tc.tile_pool(name="pad", bufs=4) as padp,
        ):
            # Batch-0 enc DMA first: nothing delays the start of the
            # transfer stream (every other DMA's descriptor-gen then hides
            # behind a running transfer).
            ets = []
            for b in range(BPC - 1):
                ets.append(ebufp.tile([P, HC, S], F16, name="et", tag="et"))

            def enc_dma(b):
                if compute_only:
                    nc.sync.dma_start(ets[b][0:1, 0:1, 0:1], enc[b, 0, 0:1, 0:1])
                else:
                    nc.sync.dma_start(
                        ets[b][:], enc[b].rearrange("c p s -> p c s")
                    )

            enc_dma(0)

            # w next: tiny (2KiB), gates the first matmul.
            w_col = constp.tile([P, HC], F16, name="w_col")
            nc.sync.dma_start(w_col[:], w_d.ap())

            enc_dma(1)

            # Output plumbing, all off the critical path:
            #  - zeros DMA'd over the output region (the scatter ADDs),
            #  - scatter indices,
            #  - descriptor PREP for the output scatter (SWDGE, Pool).
            prob = smallp.tile([P, S], F32, name="prob")
            nc.gpsimd.memset(prob[:], 0.0)
            if use_scatter:
                ztile = smallp.tile([BPC, S], F32, name="ztile")
                nc.gpsimd.memset(ztile[:], 0.0)
                nc.sync.dma_start(out_d.ap(), ztile[:])
                sidx = constp.tile([P, 1], I16, name="sidx")
                nc.sync.dma_start(sidx[:], sidx_d.ap())

                # prob is a full [128, S] tile (scatter shape contract);
                # rows >= BPC carry zeros (memset above) re-added to rows
                # 0-7 by tokens 8-15.
                # Completion sem must be the Tile-managed DMASW0 lane sem:
                # the end-of-kernel drain waits on it, and with prepare_only
                # the descriptor (fired by trigger_dma) increments it.
                nc.gpsimd.dma_scatter_add(
                    out_d.ap(),
                    prob[:].unsqueeze(1),  # [128,1,S]: 128*1 == roundup(16,128)
                    sidx[:],
                    16,
                    16,
                    S,
                    prepare_only=True,
                    sem=tc.sems.swdge_block()[0],
                )

            for b2 in range(2, BPC - 1):
                enc_dma(b2)

            # The LAST batch arrives as two s-halves (s 0:256 and 256:512,
            # i.e. j-chunks {0,1} and {2,3}): the j0/j1 half of the score
            # pipeline (matmuls, copy, transposes, exp) then overlaps the
            # second half's 1456ns transfer, shortening the exposed tail.
            SH = S // 2
            et7 = [
                ebufp.tile([P, HC, SH], F16, name="et7", tag=f"et7{h}")
                for h in range(2)
            ]
            for h in range(2):
                if compute_only:
                    nc.sync.dma_start(
                        et7[h][0:1, 0:1, 0:1], enc[BPC - 1, 0, 0:1, 0:1]
                    )
                else:
                    nc.sync.dma_start(
                        et7[h][:],
                        enc[BPC - 1].rearrange("c p s -> p c s")[
                            :, :, h * SH : (h + 1) * SH
                        ],
                    )

            # Lane-alignment padding: the end-of-scope drain waits the five
            # HWDGE lane sems in lane order, so the output DMA must be on
            # lane 4 for the other four waits to clear during its 900ns
            # sem-prop.  These four tiny re-reads of w shift it from lane 0
            # to lane 4; their transfers slot into the stream for ~7ns each.
            for _ in range(4):
                pad = padp.tile([P, HC], F16, name="pad")
                nc.sync.dma_start(pad[:], w_d.ap())

            # identity for the PE transposes, built on-device (gpsimd is
            # otherwise idle): ones everywhere, keep only where p - f == 0.
            ones_id = constp.tile([P, P], F32, name="ones_id")
            nc.gpsimd.memset(ones_id[:], 1.0)
            id_t = constp.tile([P, P], F32, name="id_t")
            nc.gpsimd.affine_select(
                out=id_t[:],
                in_=ones_id[:],
                pattern=[[-1, P]],
                compare_op=mybir.AluOpType.is_equal,
                fill=0.0,
                channel_multiplier=1,
            )

            if dma_only:
                nc.vector.tensor_copy(prob[0:BPC, :], ets[0][0:BPC, 0, 0:S])
                if use_scatter:
                    nc.gpsimd.trigger_dma(count=None)
                else:
                    nc.sync.dma_start(out_d.ap(), prob[0:BPC, :])
            else:
                # scores ps[p, j*BPC + b] = enc[b, j*128+p, :] . w_enc,
                # accumulated over the 8 h-chunks on the PE array.
                def lhs(b, j, c):
                    if b < BPC - 1:
                        return ets[b][:, c, j * P : (j + 1) * P]
                    return et7[j // 2][:, c, (j % 2) * P : (j % 2 + 1) * P]

                # Scores for j-chunks {0,1} and {2,3} accumulate in
                # SEPARATE full-bank PSUM tiles: PSUM dependency tracking is
                # bank-granular, so sharing a bank would gate the stage-1
                # copy on the very last matmul and kill the overlap.
                ps = [
                    psump.tile([P, S], F32, name=f"ps{h}") for h in range(2)
                ]
                psumT = [
                    psump.tile([BPC, S], F32, name=f"psumT{h}") for h in range(2)
                ]
                # The last batch's j2/j3 score columns accumulate in their
                # OWN bank: the bulk stage-2 copy (reading ps[1]) is then
                # gated by batch 6's matmuls, not batch 7's, and runs under
                # the final half-transfer; only a tiny strided copy of these
                # two columns stays in the exposed tail.
                psb7 = psump.tile([P, 2], F32, name="psb7")

                def score_col(b, j):
                    if b == BPC - 1 and j >= 2:
                        return psb7[:, (j - 2) : (j - 1)]
                    return ps[j // 2][:, (j % 2) * BPC + b : (j % 2) * BPC + b + 1]

                for b in range(BPC):
                    for j in range(JT):
                        for c in range(HC):
                            nc.tensor.matmul(
                                score_col(b, j),
                                lhs(b, j, c),
                                w_col[:, c : c + 1],
                                start=(c == 0),
                                stop=(c == HC - 1),
                            )

                # Two-stage tail, one stage per s-half (j-chunks {0,1} and
                # {2,3}).  Each half gets its own scores copy, transposes
                # into its OWN full-bank PSUM tile (bank-granular dep
                # tracking: sharing one bank would chain exp1 behind T3),
                # and its own exp with row-sum accumulation.  Stage 1 is
                # gated only by the first half-DMA of the last batch, so it
                # runs under the second half's transfer.
                scores = smallp.tile([P, JT * BPC], F32, name="scores")
                expt = smallp.tile([BPC, S], F32, name="expt")
                sums2 = smallp.tile([BPC, 2], F32, name="sums2")
                for h in range(2):
                    if h == 0:
                        nc.vector.tensor_copy(
                            scores[:, 0 : 2 * BPC], ps[0][:, 0 : 2 * BPC]
                        )
                    else:
                        # bulk (batches 0-6, early) ...
                        nc.vector.tensor_copy(
                            scores[:, 16:23], ps[1][:, 0 : BPC - 1]
                        )
                        nc.vector.tensor_copy(
                            scores[:, 24:31], ps[1][:, BPC : 2 * BPC - 1]
                        )
                        # ... then batch 7's two columns (after the last mms)
                        nc.vector.tensor_copy(
                            scores[:, 2 * BPC + BPC - 1 : JT * BPC : BPC], psb7[:, 0:2]
                        )
                    for j in (2 * h, 2 * h + 1):
                        nc.tensor.transpose(
                            psumT[h][:, (j % 2) * P : (j % 2 + 1) * P],
                            scores[:, j * BPC : (j + 1) * BPC],
                            id_t[:],
                        )
                    nc.scalar.activation(
                        out=expt[:, h * SH : (h + 1) * SH],
                        in_=psumT[h][:, 0 : 2 * P],
                        func=mybir.ActivationFunctionType.Exp,
                        bias=0.0,
                        scale=1.0,
                        accum_out=sums2[:, h : h + 1],
                    )

                sums = smallp.tile([BPC, 1], F32, name="sums")
                nc.vector.tensor_reduce(
                    out=sums[:],
                    in_=sums2[:],
                    axis=mybir.AxisListType.X,
                    op=mybir.AluOpType.add,
                )
                binv = smallp.tile([BPC, 1], F32, name="binv")
                nc.vector.reciprocal(binv[:], sums[:])
                nc.vector.tensor_scalar_mul(prob[0:BPC, :], expt[:], binv[:])

                if use_scatter:
                    # fire the pre-generated output descriptors (SWDGE): the
                    # trigger carries the data dep on prob, the prep did not.
                    nc.gpsimd.trigger_dma(count=None)
                    # consume the scatter's completion sem before the
                    # end-of-scope sem-range clear (race detector).
                    nc.gpsimd.wait_ge(tc.sems.swdge_block()[0], 16)
                else:
                    nc.sync.dma_start(out_d.ap(), prob[0:BPC, :])

    nc.finalize()  # Bacc: runs compile() (wait legalization, reg alloc, ...)
    return nc


def kernel(hidden, encoder_outputs, W, b, v):
    global _compiled_nc, LAST_RESULTS

    # Fold the linear layer on host (fp64 for accuracy): only the
    # encoder-input slice of W survives the softmax. Force numpy so the fold
    # never runs through a jax device backend.
    W = np.asarray(W)
    v = np.asarray(v)
    w_enc = (v.astype(np.float64) @ W[:, 2 * H :].astype(np.float64)).astype(
        np.float32
    )
    # w_col[p, c] = w_enc[c*128 + p]
    w_col = np.ascontiguousarray(w_enc.reshape(HC, P).T).astype(np.float16)
    # enc_t[b, c, p, s] = enc[b, s, c*128+p], fp16
    enc = np.asarray(encoder_outputs).astype(np.float16)
    enc_t = np.ascontiguousarray(
        enc.reshape(B, S, HC, P).transpose(0, 2, 3, 1)
    )
    # 16 scatter tokens: tokens 0-7 carry the probs; tokens 8-15 re-target
    # rows 0-7 but read prob rows 8-15, which are memset to zero on device,
    # so they add 0.  (All-valid indices keep the DMA completion semaphore
    # at its expected count of 16.)
    sidx = np.full((128, 1), -1, dtype=np.int16)
    sidx[:BPC, 0] = np.arange(BPC, dtype=np.int16)
    sidx[BPC:16, 0] = np.arange(BPC, dtype=np.int16)

    if _compiled_nc is None:
        _compiled_nc = _build_nc()

    in_maps = [
        {
            "enc_in": enc_t[c * BPC : (c + 1) * BPC],
            "w_in": w_col,
            "sidx_in": sidx,
        }
        for c in range(N_CORES)
    ]
    # The axon/PJRT path sporadically throws a transient INTERNAL error at
    # compile time; a retry has always succeeded.
    last_err = None
    for _attempt in range(3):
        try:
            LAST_RESULTS = run_bass_kernel_spmd(
                _compiled_nc, in_maps, core_ids=list(range(N_CORES))
            )
            break
        except Exception as e:  # noqa: BLE001
            last_err = e
    else:
        raise last_err
    out = np.concatenate([r["probs_out"] for r in LAST_RESULTS.results], axis=0)
    return out.astype(np.float32)


# revision 43
# speedup vs baseline: 1.0168x; 1.0000x over previous
"""Trainium2 Bass kernel for nn_Attn_58669253263845 (sparse_attention).

Reference computation:
    hidden2 = concat(hidden[0], hidden[1])                 # [B, 2H]
    attn_input = concat(bcast(hidden2), encoder_outputs)   # [B, S, 3H]
    energy = attn_input @ W.T + b                          # [B, S, H]
    scores = energy @ v                                    # [B, S]
    out = softmax(scores, axis=S)

Everything before the softmax is linear, so
    scores[b,s] = attn_input[b,s,:] . (v @ W) + v.b
                = hidden2[b,:] . w_hid + enc[b,s,:] . w_enc + v.b
The hidden/bias terms are constant per batch row and cancel in the softmax
over S.  Hence:
    out = softmax_s(enc[b,s,:] . w_enc),  w_enc = v @ W[:, 2H:3H]

The weight fold (1024x1024 matvec) is done on host in fp64; the heavy part
(64*512 dot products of length 1024 + softmax) runs on 8 NeuronCores,
data-parallel over batch (8 batches per core).

Kernel strategy (v3):
  * enc ships as fp16 (half the HBM traffic of fp32; the kernel is
    DMA-bound and the 2^-11 input rounding moves scores by ~3e-3 -- two
    orders inside the 2e-2 gate).  Host pre-transposes each batch to
    [H, S] so the contraction dim lands on SBUF partitions.
  * ONE 1MiB DMA per batch: descriptor generation (HWDGE) is a serial
    ~650ns/DMA resource, so few big DMAs keep the stream transfer-bound
    (2913ns/batch at 360B/ns).  The batch-0 DMA is issued first so no
    other descriptor-gen delays the stream start.
  * The dots run on the PE array: for each (batch, s-chunk, h-chunk) the
    128x128 enc chunk is the *stationary* operand and the matching 128-row
    slice of w_enc is a single moving column, accumulating into a
    [128, 1] PSUM column over the 8 h-chunks.  Output-free-size-1 matmuls
    leave the PE essentially idle (and immune to p-state), so the DMA
    stream is the only real cost.
  * Two-stage softmax tail: the last batch arrives as two s-half DMAs,
    and the scores for j-chunks {0,1} accumulate in their OWN full-bank
    PSUM tile (PSUM dep tracking is bank-granular), so stage 1 (copy +
    2 transposes + exp[8,256] with row-sum accumulation) runs UNDER the
    second half's 1456ns transfer.  Only stage 2 + reduce + reciprocal +
    scale + one small output DMA are exposed after the final byte.
  * Four tiny padding DMAs align the output DMA onto HWDGE lane 4: the
    end-of-scope drain waits the five lane sems in lane order, so the
    four already-satisfied waits then process during the output DMA's
    900ns sem-prop window instead of serially after it (-200ns).
  * use_scatter=True switches the output to a SWDGE scatter-add whose
    descriptors are prepared during the stream and fired by trigger_dma
    (saves ~1.1us of HWDGE gen + DGE latency in the cost model).  It is
    numerically correct in CoreSim but produced garbage on real hardware
    under the bass2jax/axon execution path, so it is OFF by default.
"""

import sys
import types

import numpy as np
import concourse.bacc as bacc
import concourse.bass as bass
import concourse.mybir as mybir
import concourse.tile as tile
from concourse.bass_utils import run_bass_kernel_spmd

# run_bass_kernel_spmd(trace=True) (e.g. via BASS_TRACE=1 in the env)
# imports antenv.axon_hooks, which does not exist in this container. Register
# a stub returning "no hook" so tracing degrades gracefully instead of
# raising ModuleNotFoundError.
try:
    import antenv.axon_hooks  # noqa: F401
except ImportError:
    try:
        import antenv

        _stub = types.ModuleType("antenv.axon_hooks")
        _stub.get_axon_ntff_profile_hook = lambda: None  # type: ignore[attr-defined]
        sys.modules["antenv.axon_hooks"] = _stub
        antenv.axon_hooks = _stub
    except ImportError:
        pass

N_CORES = 8
B, S, H = 64, 512, 1024
P = 128             # SBUF partitions
BPC = B // N_CORES  # batches per core = 8
JT = S // P         # s-chunks per batch = 4
HC = H // P         # h-chunks = 8

F32 = mybir.dt.float32
F16 = mybir.dt.float16
I16 = mybir.dt.int16

_compiled_nc = None
LAST_RESULTS = None  # BassKernelResults of the most recent run (for profiling)


def _build_nc(dma_only=False, compute_only=False, use_scatter=False, pads=4):
    """Per-core kernel: probs[BPC, S] = softmax_s(enc[BPC, S, H] @ w_enc).

    dma_only / compute_only build crippled variants for cost attribution.
    """
    # Bacc (not raw Bass): its compile() legalizes multi-wait instructions
    # into EventSemaphore waits (TRN2 allows only 1 sync wait per inst).
    nc = bacc.Bacc("TRN2", target_bir_lowering=False, debug=False)

    # enc arrives pre-transposed per batch: [BPC, HC, P, S] fp16 where
    # enc_t[b, c, p, s] = enc[b, s, c*128+p].  Batch 0 additionally ships
    # fused with w_col in "enc0w_in" [P, HC*S + HC]: one DMA covers both,
    # which drops a 56ns stream slot AND makes the output DMA the 10th
    # HWDGE DMA = lane 4 (see the lane-order note below).
    enc_d = nc.dram_tensor("enc_in", [BPC, HC, P, S], F16, kind="ExternalInput")
    enc0w_d = nc.dram_tensor("enc0w_in", [P, HC * S + HC], F16, kind="ExternalInput")
    # w_col[p, c] = w_enc[c*128 + p]
    w_d = nc.dram_tensor("w_in", [P, HC], F16, kind="ExternalInput")
    # scatter indices: row i -> output row i for i < BPC, -1 (ignored) after
    sidx_d = nc.dram_tensor("sidx_in", [P, 1], I16, kind="ExternalInput")
    out_d = nc.dram_tensor("probs_out", [BPC, S], F32, kind="ExternalOutput")

    enc = enc_d.ap()

    with tile.TileContext(nc) as tc:
        with (
            tc.tile_pool(name="const", bufs=1) as constp,
            tc.tile_pool(name="ebuf", bufs=BPC - 2) as ebufp,
            tc.tile_pool(name="ebufx", bufs=3) as ebufxp,
            tc.tile_pool(name="small", bufs=1) as smallp,
            tc.tile_pool(name="psum", bufs=1, space="PSUM") as psump,
        ):
            # Batch-0 (fused with w) first: nothing delays the start of the
            # transfer stream (every other DMA's descriptor-gen then hides
            # behind a running transfer).  Its partition rows are fully
            # contiguous 8208B in DRAM.
            et0w = ebufxp.tile([P, HC * S + HC], F16, name="et0w")
            if compute_only:
                nc.sync.dma_start(et0w[0:1, 0:1], enc0w_d.ap()[0:1, 0:1])
            else:
                nc.sync.dma_start(et0w[:], enc0w_d.ap())
            w_col = et0w[:, HC * S : HC * S + HC]

            ets = {}
            for b in range(1, BPC - 1):
                ets[b] = ebufp.tile([P, HC, S], F16, name="et", tag="et")

            def enc_dma(b):
                if compute_only:
                    nc.sync.dma_start(ets[b][0:1, 0:1, 0:1], enc[b, 0, 0:1, 0:1])
                else:
                    nc.sync.dma_start(
                        ets[b][:], enc[b].rearrange("c p s -> p c s")
                    )

            enc_dma(1)

            # Output plumbing, all off the critical path:
            #  - zeros DMA'd over the output region (the scatter ADDs),
            #  - scatter indices,
            #  - descriptor PREP for the output scatter (SWDGE, Pool).
            prob = smallp.tile([P, S], F32, name="prob")
            nc.gpsimd.memset(prob[:], 0.0)
            if use_scatter:
                ztile = smallp.tile([BPC, S], F32, name="ztile")
                nc.gpsimd.memset(ztile[:], 0.0)
                nc.sync.dma_start(out_d.ap(), ztile[:])
                sidx = constp.tile([P, 1], I16, name="sidx")
                nc.sync.dma_start(sidx[:], sidx_d.ap())

                # prob is a full [128, S] tile (scatter shape contract);
                # rows >= BPC carry zeros (memset above) re-added to rows
                # 0-7 by tokens 8-15.
                # Completion sem must be the Tile-managed DMASW0 lane sem:
                # the end-of-kernel drain waits on it, and with prepare_only
                # the descriptor (fired by trigger_dma) increments it.
                nc.gpsimd.dma_scatter_add(
                    out_d.ap(),
                    prob[:].unsqueeze(1),  # [128,1,S]: 128*1 == roundup(16,128)
                    sidx[:],
                    16,
                    16,
                    S,
                    prepare_only=True,
                    sem=tc.sems.swdge_block()[0],
                )

            for b2 in range(2, BPC - 1):
                enc_dma(b2)

            # The LAST batch arrives as two s-halves (s 0:256 and 256:512,
            # i.e. j-chunks {0,1} and {2,3}): the j0/j1 half of the score
            # pipeline (matmuls, copy, transposes, exp) then overlaps the
            # second half's 1456ns transfer, shortening the exposed tail.
            SH = S // 2
            et7 = [
                ebufxp.tile([P, HC, SH], F16, name="et7", tag=f"et7{h}")
                for h in range(2)
            ]
            for h in range(2):
                if compute_only:
                    nc.sync.dma_start(
                        et7[h][0:1, 0:1, 0:1], enc[BPC - 1, 0, 0:1, 0:1]
                    )
                else:
                    nc.sync.dma_start(
                        et7[h][:],
                        enc[BPC - 1].rearrange("c p s -> p c s")[
                            :, :, h * SH : (h + 1) * SH
                        ],
                    )

            # Lane alignment: the end-of-scope drain's five HWDGE lane
            # waits process in a fixed order; `pads` tiny DMAs shift which
            # lane the output DMA lands on so the four already-satisfied
            # waits process during its 900ns sem-prop window, not after it.
            for _ in range(pads):
                pad = ebufxp.tile([1, 1], F16, name="pad", tag="pad")
                nc.sync.dma_start(pad[:], w_d.ap()[0:1, 0:1])

            # identity for the PE transposes, built on-device (gpsimd is
            # otherwise idle): ones everywhere, keep only where p - f == 0.
            ones_id = constp.tile([P, P], F32, name="ones_id")
            nc.gpsimd.memset(ones_id[:], 1.0)
            id_t = constp.tile([P, P], F32, name="id_t")
            nc.gpsimd.affine_select(
                out=id_t[:],
                in_=ones_id[:],
                pattern=[[-1, P]],
                compare_op=mybir.AluOpType.is_equal,
                fill=0.0,
                channel_multiplier=1,
            )

            if dma_only:
                nc.vector.tensor_copy(prob[0:BPC, :], et0w[0:BPC, 0:S])
                if use_scatter:
                    nc.gpsimd.trigger_dma(count=None)
                else:
                    nc.sync.dma_start(out_d.ap(), prob[0:BPC, :])
            else:
                # scores ps[p, j*BPC + b] = enc[b, j*128+p, :] . w_enc,
                # accumulated over the 8 h-chunks on the PE array.
                def lhs(b, j, c):
                    if b == 0:
                        return et0w[:, c * S + j * P : c * S + (j + 1) * P]
                    if b < BPC - 1:
                        return ets[b][:, c, j * P : (j + 1) * P]
                    return et7[j // 2][:, c, (j % 2) * P : (j % 2 + 1) * P]

                # Scores for j-chunks {0,1} and {2,3} accumulate in
                # SEPARATE full-bank PSUM tiles: PSUM dependency tracking is
                # bank-granular, so sharing a bank would gate the stage-1
                # copy on the very last matmul and kill the overlap.
                ps = [
                    psump.tile([P, S], F32, name=f"ps{h}") for h in range(2)
                ]
                psumT = [
                    psump.tile([BPC, S], F32, name=f"psumT{h}") for h in range(2)
                ]
                # The last batch's j2/j3 score columns accumulate in their
                # OWN bank: the bulk stage-2 copy (reading ps[1]) is then
                # gated by batch 6's matmuls, not batch 7's, and runs under
                # the final half-transfer; only a tiny strided copy of these
                # two columns stays in the exposed tail.
                psb7 = psump.tile([P, 2], F32, name="psb7")

                def score_col(b, j):
                    if b == BPC - 1 and j >= 2:
                        return psb7[:, (j - 2) : (j - 1)]
                    return ps[j // 2][:, (j % 2) * BPC + b : (j % 2) * BPC + b + 1]

                for b in range(BPC):
                    for j in range(JT):
                        for c in range(HC):
                            nc.tensor.matmul(
                                score_col(b, j),
                                lhs(b, j, c),
                                w_col[:, c : c + 1],
                                start=(c == 0),
                                stop=(c == HC - 1),
                            )

                # Two-stage tail, one stage per s-half (j-chunks {0,1} and
                # {2,3}).  Each half gets its own scores copy, transposes
                # into its OWN full-bank PSUM tile (bank-granular dep
                # tracking: sharing one bank would chain exp1 behind T3),
                # and its own exp with row-sum accumulation.  Stage 1 is
                # gated only by the first half-DMA of the last batch, so it
                # runs under the second half's transfer.
                scores = smallp.tile([P, JT * BPC], F32, name="scores")
                expt = smallp.tile([BPC, S], F32, name="expt")
                sums2 = smallp.tile([BPC, 2], F32, name="sums2")
                for h in range(2):
                    if h == 0:
                        nc.vector.tensor_copy(
                            scores[:, 0 : 2 * BPC], ps[0][:, 0 : 2 * BPC]
                        )
                    else:
                        # bulk (batches 0-6, early) ...
                        nc.vector.tensor_copy(
                            scores[:, 16:23], ps[1][:, 0 : BPC - 1]
                        )
                        nc.vector.tensor_copy(
                            scores[:, 24:31], ps[1][:, BPC : 2 * BPC - 1]
                        )
                        # ... then batch 7's two columns (after the last mms)
                        nc.vector.tensor_copy(
                            scores[:, 2 * BPC + BPC - 1 : JT * BPC : BPC], psb7[:, 0:2]
                        )
                    for j in (2 * h, 2 * h + 1):
                        nc.tensor.transpose(
                            psumT[h][:, (j % 2) * P : (j % 2 + 1) * P],
                            scores[:, j * BPC : (j + 1) * BPC],
                            id_t[:],
                        )
                    nc.scalar.activation(
                        out=expt[:, h * SH : (h + 1) * SH],
                        in_=psumT[h][:, 0 : 2 * P],
                        func=mybir.ActivationFunctionType.Exp,
                        bias=0.0,
                        scale=1.0,
                        accum_out=sums2[:, h : h + 1],
                    )

                sums = smallp.tile([BPC, 1], F32, name="sums")
                nc.vector.tensor_reduce(
                    out=sums[:],
                    in_=sums2[:],
                    axis=mybir.AxisListType.X,
                    op=mybir.AluOpType.add,
                )
                binv = smallp.tile([BPC, 1], F32, name="binv")
                nc.vector.reciprocal(binv[:], sums[:])
                nc.vector.tensor_scalar_mul(prob[0:BPC, :], expt[:], binv[:])

                if use_scatter:
                    # fire the pre-generated output descriptors (SWDGE): the
                    # trigger carries the data dep on prob, the prep did not.
                    nc.gpsimd.trigger_dma(count=None)
                    # consume the scatter's completion sem before the
                    # end-of-scope sem-range clear (race detector).
                    nc.gpsimd.wait_ge(tc.sems.swdge_block()[0], 16)
                else:
                    nc.sync.dma_start(out_d.ap(), prob[0:BPC, :])

    nc.finalize()  # Bacc: runs compile() (wait legalization, reg alloc, ...)
    return nc


def kernel(hidden, encoder_outputs, W, b, v):
    global _compiled_nc, LAST_RESULTS

    # Fold the linear layer on host (fp64 for accuracy): only the
    # encoder-input slice of W survives the softmax. Force numpy so the fold
    # never runs through a jax device backend.
    W = np.asarray(W)
    v = np.asarray(v)
    w_enc = (v.astype(np.float64) @ W[:, 2 * H :].astype(np.float64)).astype(
        np.float32
    )
    # w_col[p, c] = w_enc[c*128 + p]
    w_col = np.ascontiguousarray(w_enc.reshape(HC, P).T).astype(np.float16)
    # enc_t[b, c, p, s] = enc[b, s, c*128+p], fp16
    enc = np.asarray(encoder_outputs).astype(np.float16)
    enc_t = np.ascontiguousarray(
        enc.reshape(B, S, HC, P).transpose(0, 2, 3, 1)
    )
    # 16 scatter tokens: tokens 0-7 carry the probs; tokens 8-15 re-target
    # rows 0-7 but read prob rows 8-15, which are memset to zero on device,
    # so they add 0.  (All-valid indices keep the DMA completion semaphore
    # at its expected count of 16.)
    sidx = np.full((128, 1), -1, dtype=np.int16)
    sidx[:BPC, 0] = np.arange(BPC, dtype=np.int16)
    sidx[BPC:16, 0] = np.arange(BPC, dtype=np.int16)

    if _compiled_nc is None:
        _compiled_nc = _build_nc()

    # fused batch-0 + w input: [P, HC*S + HC] per core
    def enc0w(c):
        b0 = enc_t[c * BPC]  # [HC, P, S]
        flat = np.ascontiguousarray(b0.transpose(1, 0, 2)).reshape(P, HC * S)
        return np.ascontiguousarray(np.concatenate([flat, w_col], axis=1))

    in_maps = [
        {
            "enc_in": enc_t[c * BPC : (c + 1) * BPC],
            "enc0w_in": enc0w(c),
            "w_in": w_col,
            "sidx_in": sidx,
        }
        for c in range(N_CORES)
    ]
    # The axon/PJRT path sporadically throws a transient INTERNAL error at
    # compile time; a retry has always succeeded.
    last_err = None
    for _attempt in range(3):
        try:
            LAST_RESULTS = run_bass_kernel_spmd(
                _compiled_nc, in_maps, core_ids=list(range(N_CORES))
            )
            break
        except Exception as e:  # noqa: BLE001
            last_err = e
    else:
        raise last_err
    out = np.concatenate([r["probs_out"] for r in LAST_RESULTS.results], axis=0)
    return out.astype(np.float32)


# revision 44
# speedup vs baseline: 1.0200x; 1.0032x over previous
"""Trainium2 Bass kernel for nn_Attn_58669253263845 (sparse_attention).

Reference computation:
    hidden2 = concat(hidden[0], hidden[1])                 # [B, 2H]
    attn_input = concat(bcast(hidden2), encoder_outputs)   # [B, S, 3H]
    energy = attn_input @ W.T + b                          # [B, S, H]
    scores = energy @ v                                    # [B, S]
    out = softmax(scores, axis=S)

Everything before the softmax is linear, so
    scores[b,s] = attn_input[b,s,:] . (v @ W) + v.b
                = hidden2[b,:] . w_hid + enc[b,s,:] . w_enc + v.b
The hidden/bias terms are constant per batch row and cancel in the softmax
over S.  Hence:
    out = softmax_s(enc[b,s,:] . w_enc),  w_enc = v @ W[:, 2H:3H]

The weight fold (1024x1024 matvec) is done on host in fp64; the heavy part
(64*512 dot products of length 1024 + softmax) runs on 8 NeuronCores,
data-parallel over batch (8 batches per core).

Kernel strategy (v3):
  * enc ships as fp16 (half the HBM traffic of fp32; the kernel is
    DMA-bound and the 2^-11 input rounding moves scores by ~3e-3 -- two
    orders inside the 2e-2 gate).  Host pre-transposes each batch to
    [H, S] so the contraction dim lands on SBUF partitions.
  * ONE 1MiB DMA per batch: descriptor generation (HWDGE) is a serial
    ~650ns/DMA resource, so few big DMAs keep the stream transfer-bound
    (2913ns/batch at 360B/ns).  The batch-0 DMA is issued first so no
    other descriptor-gen delays the stream start.
  * The dots run on the PE array: for each (batch, s-chunk, h-chunk) the
    128x128 enc chunk is the *stationary* operand and the matching 128-row
    slice of w_enc is a single moving column, accumulating into a
    [128, 1] PSUM column over the 8 h-chunks.  Output-free-size-1 matmuls
    leave the PE essentially idle (and immune to p-state), so the DMA
    stream is the only real cost.
  * Two-stage softmax tail: the last batch arrives as two s-half DMAs,
    and the scores for j-chunks {0,1} accumulate in their OWN full-bank
    PSUM tile (PSUM dep tracking is bank-granular), so stage 1 (copy +
    2 transposes + exp[8,256] with row-sum accumulation) runs UNDER the
    second half's 1456ns transfer.  Only stage 2 + reduce + reciprocal +
    scale + one small output DMA are exposed after the final byte.
  * Four tiny padding DMAs align the output DMA onto HWDGE lane 4: the
    end-of-scope drain waits the five lane sems in lane order, so the
    four already-satisfied waits then process during the output DMA's
    900ns sem-prop window instead of serially after it (-200ns).
  * use_scatter=True switches the output to a SWDGE scatter-add whose
    descriptors are prepared during the stream and fired by trigger_dma
    (saves ~1.1us of HWDGE gen + DGE latency in the cost model).  It is
    numerically correct in CoreSim but produced garbage on real hardware
    under the bass2jax/axon execution path, so it is OFF by default.
"""

import sys
import types

import numpy as np
import concourse.bacc as bacc
import concourse.bass as bass
import concourse.mybir as mybir
import concourse.tile as tile
from concourse.bass_utils import run_bass_kernel_spmd

# run_bass_kernel_spmd(trace=True) (e.g. via BASS_TRACE=1 in the env)
# imports antenv.axon_hooks, which does not exist in this container. Register
# a stub returning "no hook" so tracing degrades gracefully instead of
# raising ModuleNotFoundError.
try:
    import antenv.axon_hooks  # noqa: F401
except ImportError:
    try:
        import antenv

        _stub = types.ModuleType("antenv.axon_hooks")
        _stub.get_axon_ntff_profile_hook = lambda: None  # type: ignore[attr-defined]
        sys.modules["antenv.axon_hooks"] = _stub
        antenv.axon_hooks = _stub
    except ImportError:
        pass

N_CORES = 8
B, S, H = 64, 512, 1024
P = 128             # SBUF partitions
BPC = B // N_CORES  # batches per core = 8
JT = S // P         # s-chunks per batch = 4
HC = H // P         # h-chunks = 8

F32 = mybir.dt.float32
F16 = mybir.dt.float16
I16 = mybir.dt.int16

_compiled_nc = None
LAST_RESULTS = None  # BassKernelResults of the most recent run (for profiling)


def _build_nc(dma_only=False, compute_only=False, use_scatter=False, pads=6):
    """Per-core kernel: probs[BPC, S] = softmax_s(enc[BPC, S, H] @ w_enc).

    dma_only / compute_only build crippled variants for cost attribution.
    """
    # Bacc (not raw Bass): its compile() legalizes multi-wait instructions
    # into EventSemaphore waits (TRN2 allows only 1 sync wait per inst).
    nc = bacc.Bacc("TRN2", target_bir_lowering=False, debug=False)

    # enc arrives pre-transposed per batch: [BPC, HC, P, S] fp16 where
    # enc_t[b, c, p, s] = enc[b, s, c*128+p].  Batch 0 additionally ships
    # fused with w_col in "enc0w_in" [P, HC*S + HC]: one DMA covers both,
    # which drops a 56ns stream slot AND makes the output DMA the 10th
    # HWDGE DMA = lane 4 (see the lane-order note below).
    enc_d = nc.dram_tensor("enc_in", [BPC, HC, P, S], F16, kind="ExternalInput")
    enc0w_d = nc.dram_tensor("enc0w_in", [P, HC * S + HC], F16, kind="ExternalInput")
    # w_col[p, c] = w_enc[c*128 + p]
    w_d = nc.dram_tensor("w_in", [P, HC], F16, kind="ExternalInput")
    # scatter indices: row i -> output row i for i < BPC, -1 (ignored) after
    sidx_d = nc.dram_tensor("sidx_in", [P, 1], I16, kind="ExternalInput")
    out_d = nc.dram_tensor("probs_out", [BPC, S], F32, kind="ExternalOutput")

    enc = enc_d.ap()

    with tile.TileContext(nc) as tc:
        with (
            tc.tile_pool(name="const", bufs=1) as constp,
            tc.tile_pool(name="ebuf", bufs=BPC - 2) as ebufp,
            tc.tile_pool(name="ebufx", bufs=3) as ebufxp,
            tc.tile_pool(name="small", bufs=1) as smallp,
            tc.tile_pool(name="psum", bufs=1, space="PSUM") as psump,
        ):
            # Batch-0 (fused with w) first: nothing delays the start of the
            # transfer stream (every other DMA's descriptor-gen then hides
            # behind a running transfer).  Its partition rows are fully
            # contiguous 8208B in DRAM.
            et0w = ebufxp.tile([P, HC * S + HC], F16, name="et0w")
            if compute_only:
                nc.sync.dma_start(et0w[0:1, 0:1], enc0w_d.ap()[0:1, 0:1])
            else:
                nc.sync.dma_start(et0w[:], enc0w_d.ap())
            w_col = et0w[:, HC * S : HC * S + HC]

            ets = {}
            for b in range(1, BPC - 1):
                ets[b] = ebufp.tile([P, HC, S], F16, name="et", tag="et")

            def enc_dma(b):
                if compute_only:
                    nc.sync.dma_start(ets[b][0:1, 0:1, 0:1], enc[b, 0, 0:1, 0:1])
                else:
                    nc.sync.dma_start(
                        ets[b][:], enc[b].rearrange("c p s -> p c s")
                    )

            enc_dma(1)

            # Output plumbing, all off the critical path:
            #  - zeros DMA'd over the output region (the scatter ADDs),
            #  - scatter indices,
            #  - descriptor PREP for the output scatter (SWDGE, Pool).
            prob = smallp.tile([P, S], F32, name="prob")
            nc.gpsimd.memset(prob[:], 0.0)
            if use_scatter:
                ztile = smallp.tile([BPC, S], F32, name="ztile")
                nc.gpsimd.memset(ztile[:], 0.0)
                nc.sync.dma_start(out_d.ap(), ztile[:])
                sidx = constp.tile([P, 1], I16, name="sidx")
                nc.sync.dma_start(sidx[:], sidx_d.ap())

                # prob is a full [128, S] tile (scatter shape contract);
                # rows >= BPC carry zeros (memset above) re-added to rows
                # 0-7 by tokens 8-15.
                # Completion sem must be the Tile-managed DMASW0 lane sem:
                # the end-of-kernel drain waits on it, and with prepare_only
                # the descriptor (fired by trigger_dma) increments it.
                nc.gpsimd.dma_scatter_add(
                    out_d.ap(),
                    prob[:].unsqueeze(1),  # [128,1,S]: 128*1 == roundup(16,128)
                    sidx[:],
                    16,
                    16,
                    S,
                    prepare_only=True,
                    sem=tc.sems.swdge_block()[0],
                )

            for b2 in range(2, BPC - 1):
                enc_dma(b2)

            # The LAST batch arrives as two s-halves (s 0:256 and 256:512,
            # i.e. j-chunks {0,1} and {2,3}): the j0/j1 half of the score
            # pipeline (matmuls, copy, transposes, exp) then overlaps the
            # second half's 1456ns transfer, shortening the exposed tail.
            SH = S // 2
            et7 = [
                ebufxp.tile([P, HC, SH], F16, name="et7", tag=f"et7{h}")
                for h in range(2)
            ]
            for h in range(2):
                if compute_only:
                    nc.sync.dma_start(
                        et7[h][0:1, 0:1, 0:1], enc[BPC - 1, 0, 0:1, 0:1]
                    )
                else:
                    nc.sync.dma_start(
                        et7[h][:],
                        enc[BPC - 1].rearrange("c p s -> p c s")[
                            :, :, h * SH : (h + 1) * SH
                        ],
                    )

            # Lane alignment: the end-of-scope drain's five HWDGE lane
            # waits process in a fixed order; `pads` tiny DMAs shift which
            # lane the output DMA lands on so the four already-satisfied
            # waits process during its 900ns sem-prop window, not after it.
            for _ in range(pads):
                pad = ebufxp.tile([1, 1], F16, name="pad", tag="pad")
                nc.sync.dma_start(pad[:], w_d.ap()[0:1, 0:1])

            # identity for the PE transposes, built on-device (gpsimd is
            # otherwise idle): ones everywhere, keep only where p - f == 0.
            ones_id = constp.tile([P, P], F32, name="ones_id")
            nc.gpsimd.memset(ones_id[:], 1.0)
            id_t = constp.tile([P, P], F32, name="id_t")
            nc.gpsimd.affine_select(
                out=id_t[:],
                in_=ones_id[:],
                pattern=[[-1, P]],
                compare_op=mybir.AluOpType.is_equal,
                fill=0.0,
                channel_multiplier=1,
            )

            if dma_only:
                nc.vector.tensor_copy(prob[0:BPC, :], et0w[0:BPC, 0:S])
                if use_scatter:
                    nc.gpsimd.trigger_dma(count=None)
                else:
                    nc.sync.dma_start(out_d.ap(), prob[0:BPC, :])
            else:
                # scores ps[p, j*BPC + b] = enc[b, j*128+p, :] . w_enc,
                # accumulated over the 8 h-chunks on the PE array.
                def lhs(b, j, c):
                    if b == 0:
                        return et0w[:, c * S + j * P : c * S + (j + 1) * P]
                    if b < BPC - 1:
                        return ets[b][:, c, j * P : (j + 1) * P]
                    return et7[j // 2][:, c, (j % 2) * P : (j % 2 + 1) * P]

                # Scores for j-chunks {0,1} and {2,3} accumulate in
                # SEPARATE full-bank PSUM tiles: PSUM dependency tracking is
                # bank-granular, so sharing a bank would gate the stage-1
                # copy on the very last matmul and kill the overlap.
                ps = [
                    psump.tile([P, S], F32, name=f"ps{h}") for h in range(2)
                ]
                psumT = [
                    psump.tile([BPC, S], F32, name=f"psumT{h}") for h in range(2)
                ]
                # The last batch's j2/j3 score columns accumulate in their
                # OWN bank: the bulk stage-2 copy (reading ps[1]) is then
                # gated by batch 6's matmuls, not batch 7's, and runs under
                # the final half-transfer; only a tiny strided copy of these
                # two columns stays in the exposed tail.
                psb7 = psump.tile([P, 2], F32, name="psb7")

                def score_col(b, j):
                    if b == BPC - 1 and j >= 2:
                        return psb7[:, (j - 2) : (j - 1)]
                    return ps[j // 2][:, (j % 2) * BPC + b : (j % 2) * BPC + b + 1]

                for b in range(BPC):
                    for j in range(JT):
                        for c in range(HC):
                            nc.tensor.matmul(
                                score_col(b, j),
                                lhs(b, j, c),
                                w_col[:, c : c + 1],
                                start=(c == 0),
                                stop=(c == HC - 1),
                            )

                # Two-stage tail, one stage per s-half (j-chunks {0,1} and
                # {2,3}).  Each half gets its own scores copy, transposes
                # into its OWN full-bank PSUM tile (bank-granular dep
                # tracking: sharing one bank would chain exp1 behind T3),
                # and its own exp with row-sum accumulation.  Stage 1 is
                # gated only by the first half-DMA of the last batch, so it
                # runs under the second half's transfer.
                scores = smallp.tile([P, JT * BPC], F32, name="scores")
                expt = smallp.tile([BPC, S], F32, name="expt")
                sums2 = smallp.tile([BPC, 2], F32, name="sums2")
                for h in range(2):
                    if h == 0:
                        nc.vector.tensor_copy(
                            scores[:, 0 : 2 * BPC], ps[0][:, 0 : 2 * BPC]
                        )
                    else:
                        # bulk (batches 0-6, early) ...
                        nc.vector.tensor_copy(
                            scores[:, 16:23], ps[1][:, 0 : BPC - 1]
                        )
                        nc.vector.tensor_copy(
                            scores[:, 24:31], ps[1][:, BPC : 2 * BPC - 1]
                        )
                        # ... then batch 7's two columns (after the last mms)
                        nc.vector.tensor_copy(
                            scores[:, 2 * BPC + BPC - 1 : JT * BPC : BPC], psb7[:, 0:2]
                        )
                    for j in (2 * h, 2 * h + 1):
                        nc.tensor.transpose(
                            psumT[h][:, (j % 2) * P : (j % 2 + 1) * P],
                            scores[:, j * BPC : (j + 1) * BPC],
                            id_t[:],
                        )
                    nc.scalar.activation(
                        out=expt[:, h * SH : (h + 1) * SH],
                        in_=psumT[h][:, 0 : 2 * P],
                        func=mybir.ActivationFunctionType.Exp,
                        bias=0.0,
                        scale=1.0,
                        accum_out=sums2[:, h : h + 1],
                    )

                sums = smallp.tile([BPC, 1], F32, name="sums")
                nc.vector.tensor_reduce(
                    out=sums[:],
                    in_=sums2[:],
                    axis=mybir.AxisListType.X,
                    op=mybir.AluOpType.add,
                )
                binv = smallp.tile([BPC, 1], F32, name="binv")
                nc.vector.reciprocal(binv[:], sums[:])
                nc.vector.tensor_scalar_mul(prob[0:BPC, :], expt[:], binv[:])

                if use_scatter:
                    # fire the pre-generated output descriptors (SWDGE): the
                    # trigger carries the data dep on prob, the prep did not.
                    nc.gpsimd.trigger_dma(count=None)
                    # consume the scatter's completion sem before the
                    # end-of-scope sem-range clear (race detector).
                    nc.gpsimd.wait_ge(tc.sems.swdge_block()[0], 16)
                else:
                    nc.sync.dma_start(out_d.ap(), prob[0:BPC, :])

    nc.finalize()  # Bacc: runs compile() (wait legalization, reg alloc, ...)
    return nc


def kernel(hidden, encoder_outputs, W, b, v):
    global _compiled_nc, LAST_RESULTS

    # Fold the linear layer on host (fp64 for accuracy): only the
    # encoder-input slice of W survives the softmax. Force numpy so the fold
    # never runs through a jax device backend.
    W = np.asarray(W)
    v = np.asarray(v)
    w_enc = (v.astype(np.float64) @ W[:, 2 * H :].astype(np.float64)).astype(
        np.float32
    )
    # w_col[p, c] = w_enc[c*128 + p]
    w_col = np.ascontiguousarray(w_enc.reshape(HC, P).T).astype(np.float16)
    # enc_t[b, c, p, s] = enc[b, s, c*128+p], fp16
    enc = np.asarray(encoder_outputs).astype(np.float16)
    enc_t = np.ascontiguousarray(
        enc.reshape(B, S, HC, P).transpose(0, 2, 3, 1)
    )
    # 16 scatter tokens: tokens 0-7 carry the probs; tokens 8-15 re-target
    # rows 0-7 but read prob rows 8-15, which are memset to zero on device,
    # so they add 0.  (All-valid indices keep the DMA completion semaphore
    # at its expected count of 16.)
    sidx = np.full((128, 1), -1, dtype=np.int16)
    sidx[:BPC, 0] = np.arange(BPC, dtype=np.int16)
    sidx[BPC:16, 0] = np.arange(BPC, dtype=np.int16)

    if _compiled_nc is None:
        _compiled_nc = _build_nc()

    # fused batch-0 + w input: [P, HC*S + HC] per core
    def enc0w(c):
        b0 = enc_t[c * BPC]  # [HC, P, S]
        flat = np.ascontiguousarray(b0.transpose(1, 0, 2)).reshape(P, HC * S)
        return np.ascontiguousarray(np.concatenate([flat, w_col], axis=1))

    in_maps = [
        {
            "enc_in": enc_t[c * BPC : (c + 1) * BPC],
            "enc0w_in": enc0w(c),
            "w_in": w_col,
            "sidx_in": sidx,
        }
        for c in range(N_CORES)
    ]
    # The axon/PJRT path sporadically throws a transient INTERNAL error at
    # compile time; a retry has always succeeded.
    last_err = None
    for _attempt in range(3):
        try:
            LAST_RESULTS = run_bass_kernel_spmd(
                _compiled_nc, in_maps, core_ids=list(range(N_CORES))
            )
            break
        except Exception as e:  # noqa: BLE001
            last_err = e
    else:
        raise last_err
    out = np.concatenate([r["probs_out"] for r in LAST_RESULTS.results], axis=0)
    return out.astype(np.float32)


# revision 46
# speedup vs baseline: 1.0241x; 1.0039x over previous
"""Trainium2 Bass kernel for nn_Attn_58669253263845 (sparse_attention).

Reference computation:
    hidden2 = concat(hidden[0], hidden[1])                 # [B, 2H]
    attn_input = concat(bcast(hidden2), encoder_outputs)   # [B, S, 3H]
    energy = attn_input @ W.T + b                          # [B, S, H]
    scores = energy @ v                                    # [B, S]
    out = softmax(scores, axis=S)

Everything before the softmax is linear, so
    scores[b,s] = attn_input[b,s,:] . (v @ W) + v.b
                = hidden2[b,:] . w_hid + enc[b,s,:] . w_enc + v.b
The hidden/bias terms are constant per batch row and cancel in the softmax
over S.  Hence:
    out = softmax_s(enc[b,s,:] . w_enc),  w_enc = v @ W[:, 2H:3H]

The weight fold (1024x1024 matvec) is done on host in fp64; the heavy part
(64*512 dot products of length 1024 + softmax) runs on 8 NeuronCores,
data-parallel over batch (8 batches per core).

Kernel strategy (v3):
  * enc ships as fp16 (half the HBM traffic of fp32; the kernel is
    DMA-bound and the 2^-11 input rounding moves scores by ~3e-3 -- two
    orders inside the 2e-2 gate).  Host pre-transposes each batch to
    [H, S] so the contraction dim lands on SBUF partitions.
  * ONE 1MiB DMA per batch: descriptor generation (HWDGE) is a serial
    ~650ns/DMA resource, so few big DMAs keep the stream transfer-bound
    (2913ns/batch at 360B/ns).  The batch-0 DMA is issued first so no
    other descriptor-gen delays the stream start.
  * The dots run on the PE array: for each (batch, s-chunk, h-chunk) the
    128x128 enc chunk is the *stationary* operand and the matching 128-row
    slice of w_enc is a single moving column, accumulating into a
    [128, 1] PSUM column over the 8 h-chunks.  Output-free-size-1 matmuls
    leave the PE essentially idle (and immune to p-state), so the DMA
    stream is the only real cost.
  * Two-stage softmax tail: the last batch arrives as two s-half DMAs,
    and the scores for j-chunks {0,1} accumulate in their OWN full-bank
    PSUM tile (PSUM dep tracking is bank-granular), so stage 1 (copy +
    2 transposes + exp[8,256] with row-sum accumulation) runs UNDER the
    second half's 1456ns transfer.  Only stage 2 + reduce + reciprocal +
    scale + one small output DMA are exposed after the final byte.
  * Six tiny padding DMAs (count swept empirically in TimelineSim)
    align the output DMA's HWDGE lane so the end-of-scope drain's four
    already-satisfied lane waits process during the output DMA's 900ns
    sem-prop window instead of serially after it (-200ns).
  * use_scatter=True switches the output to a SWDGE scatter-add whose
    descriptors are prepared during the stream and fired by trigger_dma
    (saves ~1.1us of HWDGE gen + DGE latency in the cost model).  It is
    numerically correct in CoreSim but produced garbage on real hardware
    under the bass2jax/axon execution path, so it is OFF by default.
"""

import sys
import types

import numpy as np
import concourse.bacc as bacc
import concourse.bass as bass
import concourse.mybir as mybir
import concourse.tile as tile
from concourse.bass_utils import run_bass_kernel_spmd

# run_bass_kernel_spmd(trace=True) (e.g. via BASS_TRACE=1 in the env)
# imports antenv.axon_hooks, which does not exist in this container. Register
# a stub returning "no hook" so tracing degrades gracefully instead of
# raising ModuleNotFoundError.
try:
    import antenv.axon_hooks  # noqa: F401
except ImportError:
    try:
        import antenv

        _stub = types.ModuleType("antenv.axon_hooks")
        _stub.get_axon_ntff_profile_hook = lambda: None  # type: ignore[attr-defined]
        sys.modules["antenv.axon_hooks"] = _stub
        antenv.axon_hooks = _stub
    except ImportError:
        pass

N_CORES = 8
B, S, H = 64, 512, 1024
P = 128             # SBUF partitions
BPC = B // N_CORES  # batches per core = 8
JT = S // P         # s-chunks per batch = 4
HC = H // P         # h-chunks = 8

F32 = mybir.dt.float32
F16 = mybir.dt.float16
I16 = mybir.dt.int16

_compiled_nc = None
LAST_RESULTS = None  # BassKernelResults of the most recent run (for profiling)


def _build_nc(dma_only=False, compute_only=False, use_scatter=False, pads=6):
    """Per-core kernel: probs[BPC, S] = softmax_s(enc[BPC, S, H] @ w_enc).

    dma_only / compute_only build crippled variants for cost attribution.
    """
    # Bacc (not raw Bass): its compile() legalizes multi-wait instructions
    # into EventSemaphore waits (TRN2 allows only 1 sync wait per inst).
    nc = bacc.Bacc("TRN2", target_bir_lowering=False, debug=False)

    # enc arrives pre-transposed per batch: [BPC, HC, P, S] fp16 where
    # enc_t[b, c, p, s] = enc[b, s, c*128+p].  Batch 0 additionally ships
    # fused with w_col in "enc0w_in" [P, HC*S + HC]: one DMA covers both,
    # which drops a 56ns stream slot AND makes the output DMA the 10th
    # HWDGE DMA = lane 4 (see the lane-order note below).
    enc_d = nc.dram_tensor("enc_in", [BPC, HC, P, S], F16, kind="ExternalInput")
    enc0w_d = nc.dram_tensor("enc0w_in", [P, HC * S + HC], F16, kind="ExternalInput")
    # w_col[p, c] = w_enc[c*128 + p]
    w_d = nc.dram_tensor("w_in", [P, HC], F16, kind="ExternalInput")
    # scatter indices: row i -> output row i for i < BPC, -1 (ignored) after
    sidx_d = nc.dram_tensor("sidx_in", [P, 1], I16, kind="ExternalInput")
    out_d = nc.dram_tensor("probs_out", [BPC, S], F32, kind="ExternalOutput")

    enc = enc_d.ap()

    with tile.TileContext(nc) as tc:
        with (
            tc.tile_pool(name="const", bufs=1) as constp,
            tc.tile_pool(name="ebuf", bufs=BPC - 2) as ebufp,
            tc.tile_pool(name="ebufx", bufs=3) as ebufxp,
            tc.tile_pool(name="small", bufs=1) as smallp,
            tc.tile_pool(name="psum", bufs=1, space="PSUM") as psump,
        ):
            # Batch-0 (fused with w) first: nothing delays the start of the
            # transfer stream (every other DMA's descriptor-gen then hides
            # behind a running transfer).  Its partition rows are fully
            # contiguous 8208B in DRAM.
            et0w = ebufxp.tile([P, HC * S + HC], F16, name="et0w")
            if compute_only:
                nc.sync.dma_start(et0w[0:1, 0:1], enc0w_d.ap()[0:1, 0:1])
            else:
                nc.sync.dma_start(et0w[:], enc0w_d.ap())
            w_col = et0w[:, HC * S : HC * S + HC]

            ets = {}
            for b in range(1, BPC - 1):
                ets[b] = ebufp.tile([P, HC, S], F16, name="et", tag="et")

            def enc_dma(b):
                if compute_only:
                    nc.sync.dma_start(ets[b][0:1, 0:1, 0:1], enc[b, 0, 0:1, 0:1])
                else:
                    nc.sync.dma_start(
                        ets[b][:], enc[b].rearrange("c p s -> p c s")
                    )

            enc_dma(1)

            # Output plumbing, all off the critical path:
            #  - zeros DMA'd over the output region (the scatter ADDs),
            #  - scatter indices,
            #  - descriptor PREP for the output scatter (SWDGE, Pool).
            prob = smallp.tile([P, S], F32, name="prob")
            nc.gpsimd.memset(prob[:], 0.0)
            if use_scatter:
                ztile = smallp.tile([BPC, S], F32, name="ztile")
                nc.gpsimd.memset(ztile[:], 0.0)
                nc.sync.dma_start(out_d.ap(), ztile[:])
                sidx = constp.tile([P, 1], I16, name="sidx")
                nc.sync.dma_start(sidx[:], sidx_d.ap())

                # prob is a full [128, S] tile (scatter shape contract);
                # rows >= BPC carry zeros (memset above) re-added to rows
                # 0-7 by tokens 8-15.
                # Completion sem must be the Tile-managed DMASW0 lane sem:
                # the end-of-kernel drain waits on it, and with prepare_only
                # the descriptor (fired by trigger_dma) increments it.
                nc.gpsimd.dma_scatter_add(
                    out_d.ap(),
                    prob[:].unsqueeze(1),  # [128,1,S]: 128*1 == roundup(16,128)
                    sidx[:],
                    16,
                    16,
                    S,
                    prepare_only=True,
                    sem=tc.sems.swdge_block()[0],
                )

            for b2 in range(2, BPC - 1):
                enc_dma(b2)

            # The LAST batch arrives as two s-halves (s 0:256 and 256:512,
            # i.e. j-chunks {0,1} and {2,3}): the j0/j1 half of the score
            # pipeline (matmuls, copy, transposes, exp) then overlaps the
            # second half's 1456ns transfer, shortening the exposed tail.
            SH = S // 2
            et7 = [
                ebufxp.tile([P, HC, SH], F16, name="et7", tag=f"et7{h}")
                for h in range(2)
            ]
            for h in range(2):
                if compute_only:
                    nc.sync.dma_start(
                        et7[h][0:1, 0:1, 0:1], enc[BPC - 1, 0, 0:1, 0:1]
                    )
                else:
                    nc.sync.dma_start(
                        et7[h][:],
                        enc[BPC - 1].rearrange("c p s -> p c s")[
                            :, :, h * SH : (h + 1) * SH
                        ],
                    )

            # Lane alignment: the end-of-scope drain's five HWDGE lane
            # waits process in a fixed order; `pads` tiny DMAs shift which
            # lane the output DMA lands on so the four already-satisfied
            # waits process during its 900ns sem-prop window, not after it.
            for _ in range(pads):
                pad = ebufxp.tile([1, 1], F16, name="pad", tag="pad")
                nc.sync.dma_start(pad[:], w_d.ap()[0:1, 0:1])

            # identity for the PE transposes, built on-device (gpsimd is
            # otherwise idle): ones everywhere, keep only where p - f == 0.
            ones_id = constp.tile([P, P], F32, name="ones_id")
            nc.gpsimd.memset(ones_id[:], 1.0)
            id_t = constp.tile([P, P], F32, name="id_t")
            nc.gpsimd.affine_select(
                out=id_t[:],
                in_=ones_id[:],
                pattern=[[-1, P]],
                compare_op=mybir.AluOpType.is_equal,
                fill=0.0,
                channel_multiplier=1,
            )

            if dma_only:
                nc.vector.tensor_copy(prob[0:BPC, :], et0w[0:BPC, 0:S])
                if use_scatter:
                    nc.gpsimd.trigger_dma(count=None)
                else:
                    nc.sync.dma_start(out_d.ap(), prob[0:BPC, :])
            else:
                # scores ps[p, j*BPC + b] = enc[b, j*128+p, :] . w_enc,
                # accumulated over the 8 h-chunks on the PE array.
                def lhs(b, j, c):
                    if b == 0:
                        return et0w[:, c * S + j * P : c * S + (j + 1) * P]
                    if b < BPC - 1:
                        return ets[b][:, c, j * P : (j + 1) * P]
                    return et7[j // 2][:, c, (j % 2) * P : (j % 2 + 1) * P]

                # Scores for j-chunks {0,1} and {2,3} accumulate in
                # SEPARATE full-bank PSUM tiles: PSUM dependency tracking is
                # bank-granular, so sharing a bank would gate the stage-1
                # copy on the very last matmul and kill the overlap.
                ps = [
                    psump.tile([P, S], F32, name=f"ps{h}") for h in range(2)
                ]
                psumT = [
                    psump.tile([BPC, S], F32, name=f"psumT{h}") for h in range(2)
                ]
                # The last batch's j2/j3 score columns accumulate in their
                # OWN bank: the bulk stage-2 copy (reading ps[1]) is then
                # gated by batch 6's matmuls, not batch 7's, and runs under
                # the final half-transfer; only a tiny strided copy of these
                # two columns stays in the exposed tail.
                psb7 = psump.tile([P, 2], F32, name="psb7")

                def score_col(b, j):
                    if b == BPC - 1 and j >= 2:
                        return psb7[:, (j - 2) : (j - 1)]
                    return ps[j // 2][:, (j % 2) * BPC + b : (j % 2) * BPC + b + 1]

                for b in range(BPC):
                    for j in range(JT):
                        for c in range(HC):
                            nc.tensor.matmul(
                                score_col(b, j),
                                lhs(b, j, c),
                                w_col[:, c : c + 1],
                                start=(c == 0),
                                stop=(c == HC - 1),
                            )

                # Two-stage tail, one stage per s-half (j-chunks {0,1} and
                # {2,3}).  Each half gets its own scores copy, transposes
                # into its OWN full-bank PSUM tile (bank-granular dep
                # tracking: sharing one bank would chain exp1 behind T3),
                # and its own exp with row-sum accumulation.  Stage 1 is
                # gated only by the first half-DMA of the last batch, so it
                # runs under the second half's transfer.
                scores = smallp.tile([P, JT * BPC], F32, name="scores")
                expt = smallp.tile([BPC, S], F32, name="expt")
                sums2 = smallp.tile([BPC, 2], F32, name="sums2")
                for h in range(2):
                    if h == 0:
                        nc.vector.tensor_copy(
                            scores[:, 0 : 2 * BPC], ps[0][:, 0 : 2 * BPC]
                        )
                    else:
                        # bulk (batches 0-6, early) ...
                        nc.vector.tensor_copy(
                            scores[:, 16:23], ps[1][:, 0 : BPC - 1]
                        )
                        nc.vector.tensor_copy(
                            scores[:, 24:31], ps[1][:, BPC : 2 * BPC - 1]
                        )
                        # ... then batch 7's two columns (after the last mms)
                        nc.vector.tensor_copy(
                            scores[:, 2 * BPC + BPC - 1 : JT * BPC : BPC], psb7[:, 0:2]
                        )
                    for j in (2 * h, 2 * h + 1):
                        nc.tensor.transpose(
                            psumT[h][:, (j % 2) * P : (j % 2 + 1) * P],
                            scores[:, j * BPC : (j + 1) * BPC],
                            id_t[:],
                        )
                    nc.scalar.activation(
                        out=expt[:, h * SH : (h + 1) * SH],
                        in_=psumT[h][:, 0 : 2 * P],
                        func=mybir.ActivationFunctionType.Exp,
                        bias=0.0,
                        scale=1.0,
                        accum_out=sums2[:, h : h + 1],
                    )

                sums = smallp.tile([BPC, 1], F32, name="sums")
                # free-size-1 add beats a free-size-2 reduce on the DVE
                nc.vector.tensor_add(sums[:], sums2[:, 0:1], sums2[:, 1:2])
                binv = smallp.tile([BPC, 1], F32, name="binv")
                nc.vector.reciprocal(binv[:], sums[:])
                nc.vector.tensor_scalar_mul(prob[0:BPC, :], expt[:], binv[:])

                if use_scatter:
                    # fire the pre-generated output descriptors (SWDGE): the
                    # trigger carries the data dep on prob, the prep did not.
                    nc.gpsimd.trigger_dma(count=None)
                    # consume the scatter's completion sem before the
                    # end-of-scope sem-range clear (race detector).
                    nc.gpsimd.wait_ge(tc.sems.swdge_block()[0], 16)
                else:
                    nc.sync.dma_start(out_d.ap(), prob[0:BPC, :])

    nc.finalize()  # Bacc: runs compile() (wait legalization, reg alloc, ...)
    return nc


def kernel(hidden, encoder_outputs, W, b, v):
    global _compiled_nc, LAST_RESULTS

    # Fold the linear layer on host (fp64 for accuracy): only the
    # encoder-input slice of W survives the softmax. Force numpy so the fold
    # never runs through a jax device backend.
    W = np.asarray(W)
    v = np.asarray(v)
    w_enc = (v.astype(np.float64) @ W[:, 2 * H :].astype(np.float64)).astype(
        np.float32
    )
    # w_col[p, c] = w_enc[c*128 + p]
    w_col = np.ascontiguousarray(w_enc.reshape(HC, P).T).astype(np.float16)
    # enc_t[b, c, p, s] = enc[b, s, c*128+p], fp16
    enc = np.asarray(encoder_outputs).astype(np.float16)
    enc_t = np.ascontiguousarray(
        enc.reshape(B, S, HC, P).transpose(0, 2, 3, 1)
    )
    # 16 scatter tokens: tokens 0-7 carry the probs; tokens 8-15 re-target
    # rows 0-7 but read prob rows 8-15, which are memset to zero on device,
    # so they add 0.  (All-valid indices keep the DMA completion semaphore
    # at its expected count of 16.)
    sidx = np.full((128, 1), -1, dtype=np.int16)
    sidx[:BPC, 0] = np.arange(BPC, dtype=np.int16)
    sidx[BPC:16, 0] = np.arange(BPC, dtype=np.int16)

    if _compiled_nc is None:
        _compiled_nc = _build_nc()

    # fused batch-0 + w input: [P, HC*S + HC] per core
    def enc0w(c):
        b0 = enc_t[c * BPC]  # [HC, P, S]
        flat = np.ascontiguousarray(b0.transpose(1, 0, 2)).reshape(P, HC * S)
        return np.ascontiguousarray(np.concatenate([flat, w_col], axis=1))

    in_maps = [
        {
            "enc_in": enc_t[c * BPC : (c + 1) * BPC],
            "enc0w_in": enc0w(c),
            "w_in": w_col,
            "sidx_in": sidx,
        }
        for c in range(N_CORES)
    ]
    # The axon/PJRT path sporadically throws a transient INTERNAL error at
    # compile time; a retry has always succeeded.
    last_err = None
    for _attempt in range(3):
        try:
            LAST_RESULTS = run_bass_kernel_spmd(
                _compiled_nc, in_maps, core_ids=list(range(N_CORES))
            )
            break
        except Exception as e:  # noqa: BLE001
            last_err = e
    else:
        raise last_err
    out = np.concatenate([r["probs_out"] for r in LAST_RESULTS.results], axis=0)
    return out.astype(np.float32)
